# revision 1
# baseline (speedup 1.0000x reference)
"""Trainium2 Bass kernel for nn_DescriptorContrastiveLoss.

Contract: kernel(**inputs) takes FULL inputs (as produced by
reference.setup_inputs()) and returns the FULL scalar output.

Sharding: data-parallel over (batch, row-half): core c handles batch c//2,
row-half c%2.  Each core:
  - resizes canonical volumes (trilinear + antialias, exact jax weights)
    on the tensor engine (3 separable fp32 contractions with DRAM bounces),
  - computes s[n,m] = 2<a_n,b_m> - |b_m|^2 via K=4 augmented fp32 matmuls
    (argmax_m s == argmin_m d2),
  - finds per-row argmax with DVE tensor_tensor_reduce (value) +
    max_index (index) reading PSUM directly,
  - gathers matched target descriptors with an indirect DMA row-gather,
  - computes cosine similarities and a local sum.
Host combines the 8 partial sums into the final scalar loss.
"""
import sys

sys.path.insert(0, '/opt/trn_rl_repo')

import numpy as np
from contextlib import ExitStack

import concourse.bass as bass
import concourse.tile as tile
import concourse.bacc as bacc
import concourse.mybir as mybir
from concourse._compat import with_exitstack
from concourse.bass_utils import run_bass_kernel_spmd

F32 = mybir.dt.float32
U32 = mybir.dt.uint32
ALU = mybir.AluOpType
ACTF = mybir.ActivationFunctionType

B = 4
C = 3
D = 64          # input volume side
S0, S1 = 16, 8  # stage output sides
N0, N1 = S0 ** 3, S1 ** 3   # 4096, 512
CD = 32         # descriptor channels
NCORES = 8
NEG = -3.0e38


def _resize_weights(in_size: int, out_size: int) -> np.ndarray:
    """fp32-faithful replica of jax.image resize weights (triangle kernel,
    antialias=True, translation=0).  Returns [in_size, out_size]."""
    scale = out_size / in_size
    inv_scale = np.float32(1.0 / scale)
    kernel_scale = np.float32(max(1.0 / scale, 1.0))
    sample_f = ((np.arange(out_size, dtype=np.float32) + np.float32(0.5))
                * inv_scale - np.float32(0.5))
    x = np.abs(sample_f[None, :]
               - np.arange(in_size, dtype=np.float32)[:, None]) / kernel_scale
    w = np.maximum(np.float32(0), np.float32(1) - x).astype(np.float32)
    tot = w.sum(axis=0, keepdims=True, dtype=np.float32)
    w = np.where(np.abs(tot) > 1000.0 * float(np.finfo(np.float32).eps),
                 w / np.where(tot != 0, tot, 1), 0).astype(np.float32)
    valid = (sample_f >= -0.5) & (sample_f <= in_size - 0.5)
    return np.where(valid[None, :], w, 0).astype(np.float32)


# d-slice of the source volume needed per half (with filter support halo)
_SRC_D0 = {0: 0, 1: 28}
_SRC_DN = 36


@with_exitstack
def _kern(ctx: ExitStack, tc: tile.TileContext, io: dict):
    nc = tc.nc
    ct, cs = io['ct'], io['cs']
    wdt, wds, wh0, wh1 = io['wdt'], io['wds'], io['wh0'], io['wh1']
    wwb0, wwa0, wwb1, wwa1 = io['wwb0'], io['wwa0'], io['wwb1'], io['wwa1']
    td0T, sd0T, td1T, sd1T = io['td0T'], io['sd0T'], io['td1T'], io['sd1T']
    ident, out = io['ident'], io['out']
    y1t0, y1t1 = io['y1t0'], io['y1t1']
    y1s0, y1s1 = io['y1s0'], io['y1s1']
    y2t0, y2t1, y2s0, y2s1 = io['y2t0'], io['y2t1'], io['y2s0'], io['y2s1']
    baug_d, b1aug_d = io['baug'], io['b1aug']
    idx0_d, idx1_d = io['idx0_d'], io['idx1_d']
    aaug3_d, a1aug3_d = io['aaug3'], io['a1aug3']

    dbg = io.get('_dbg', 0)
    consts = ctx.enter_context(tc.tile_pool(name="consts", bufs=1))
    ident_sb = consts.tile([128, 128], F32)
    nc.sync.dma_start(ident_sb[:], ident)

    # ---------------- Phase R: resize ----------------
    with tc.tile_pool(name="rsbuf", bufs=1) as rp, \
         tc.tile_pool(name="rw", bufs=1) as rw, \
         tc.tile_pool(name="l2in", bufs=1) as l2p, \
         tc.tile_pool(name="l3in", bufs=2) as l3p, \
         tc.tile_pool(name="l3tr", bufs=2) as l3t, \
         tc.tile_pool(name="sq", bufs=1) as sqp, \
         tc.tile_pool(name="psl1", bufs=2, space="PSUM") as psl1, \
         tc.tile_pool(name="psl2", bufs=2, space="PSUM") as psl2, \
         tc.tile_pool(name="pstr", bufs=2, space="PSUM") as pstr, \
         tc.tile_pool(name="psl3", bufs=2, space="PSUM") as psl3:

        wdt_sb = rw.tile([64, 24], F32)
        nc.sync.dma_start(wdt_sb[:], wdt)
        wds_sb = rw.tile([_SRC_DN, 12], F32)
        nc.sync.dma_start(wds_sb[:], wds)
        wh0_sb = rw.tile([64, S0], F32)
        nc.sync.dma_start(wh0_sb[:], wh0)
        wh1_sb = rw.tile([64, S1], F32)
        nc.sync.dma_start(wh1_sb[:], wh1)
        wwb0_sb = rw.tile([64, S0], F32)
        nc.sync.dma_start(wwb0_sb[:], wwb0)
        wwa0_sb = rw.tile([64, S0], F32)
        nc.sync.dma_start(wwa0_sb[:], wwa0)
        wwb1_sb = rw.tile([64, S1], F32)
        nc.sync.dma_start(wwb1_sb[:], wwb1)
        wwa1_sb = rw.tile([64, S1], F32)
        nc.sync.dma_start(wwa1_sb[:], wwa1)

        # L1: contract d (both stages at once; weights stacked in cols)
        ct_sb = rp.tile([64, C * D * D], F32)
        nc.sync.dma_start(ct_sb[:].rearrange("d (c h w) -> d c h w", c=C, h=D),
                          ct.rearrange("c d h w -> d c h w"))
        cs_sb = rp.tile([_SRC_DN, C * D * D], F32)
        nc.sync.dma_start(cs_sb[:].rearrange("d (c h w) -> d c h w", c=C, h=D),
                          cs.rearrange("c d h w -> d c h w"))

        y1t0v = y1t0.rearrange("c do h w -> do c h w")
        y1t1v = y1t1.rearrange("c do h w -> do c h w")
        y1s0v = y1s0.rearrange("c do h w -> do c h w")
        y1s1v = y1s1.rearrange("c do h w -> do c h w")
        nchunk = C * D * D // 512  # 24
        for k in range(nchunk):
            sl = slice(512 * k, 512 * (k + 1))
            ck, hlo = k // 8, (k % 8) * 8
            p1 = psl1.tile([24, 512], F32, tag="p1")
            nc.tensor.matmul(p1[:], wdt_sb[:], ct_sb[:, sl],
                             start=True, stop=True)
            s1t = l3t.tile([24, 512], F32, tag="s1t")
            nc.scalar.copy(s1t[:], p1[:])
            s1tv = s1t[:].rearrange("p (h w) -> p h w", h=8)
            nc.sync.dma_start(y1t0v[:, ck, hlo:hlo + 8, :], s1tv[0:S0])
            nc.sync.dma_start(y1t1v[:, ck, hlo:hlo + 8, :], s1tv[S0:24])
            p1s = psl1.tile([24, 512], F32, tag="p1")
            nc.tensor.matmul(p1s[0:12, :], wds_sb[:], cs_sb[:, sl],
                             start=True, stop=True)
            s1s = l3t.tile([12, 512], F32, tag="s1s")
            nc.scalar.copy(s1s[:], p1s[0:12, :])
            s1sv = s1s[:].rearrange("p (h w) -> p h w", h=8)
            nc.sync.dma_start(y1s0v[:, ck, hlo:hlo + 8, :], s1sv[0:S1])
            nc.sync.dma_start(y1s1v[:, ck, hlo:hlo + 8, :], s1sv[S1:12])

        # L2: contract h.  rhs free order (c, do, w).
        l2_jobs = [
            # (src rearranged DRAM ap, lhsT, out dram, n_do, n_ho)
            (y1t0.rearrange("c do h w -> h c do w"), wh0_sb, y2t0, S0, S0),
            (y1t1.rearrange("c do h w -> h c do w"), wh1_sb, y2t1, S1, S1),
            (y1s0.rearrange("c do h w -> h c do w"), wh0_sb, y2s0, S1, S0),
            (y1s1.rearrange("c do h w -> h c do w"), wh1_sb, y2s1, 4, S1),
        ]
        for src, lhsT, dst, ndo, nho in l2_jobs:
            free = C * ndo * D
            t_in = l2p.tile([64, free], F32, tag="l2in")
            nc.sync.dma_start(
                t_in[:].rearrange("h (c do w) -> h c do w", c=C, do=ndo), src)
            dstv = dst.rearrange("c do ho w -> ho (c do) w")
            for k in range((free + 511) // 512):
                lo = 512 * k
                hi = min(free, lo + 512)
                p2 = psl2.tile([nho, 512], F32, tag="p2")
                nc.tensor.matmul(p2[:, 0:hi - lo], lhsT[:], t_in[:, lo:hi],
                                 start=True, stop=True)
                s2t = l3t.tile([nho, 512], F32, tag="s2t")
                nc.scalar.copy(s2t[:, 0:hi - lo], p2[:, 0:hi - lo])
                nc.sync.dma_start(dstv[:, lo // 64:hi // 64, :],
                                  s2t[:, 0:hi - lo])

        # L3: contract w.  chunks of <=128 rows of (c,do,ho), transpose, matmul.
        def l3(dst_flat, src, nrows, wout, w_sb, b2_sink, crows=128):
            srcv = src.rearrange("c do ho w -> (c do ho) w")
            sqs = None
            if b2_sink is not None:
                sqs = sqp.tile([128, wout * ((nrows + crows - 1) // crows)],
                               F32, tag="sqs" + str(wout))
            nch = (nrows + crows - 1) // crows
            for j in range(nch):
                lo = crows * j
                hi = min(nrows, lo + crows)
                n = hi - lo
                t_in = l3p.tile([128, 64], F32, tag="l3in")
                nc.sync.dma_start(t_in[0:n, :], srcv[lo:hi, :])
                ptr = pstr.tile([64, 128], F32, tag="ptr")
                nc.tensor.transpose(ptr[:, 0:n], t_in[0:n, :], ident_sb[0:n, 0:n])
                tr = l3t.tile([64, 128], F32, tag="l3tr")
                nc.scalar.copy(tr[:, 0:n], ptr[:, 0:n])
                p3 = psl3.tile([128, S0], F32, tag="p3")
                nc.tensor.matmul(p3[0:n, 0:wout], tr[:, 0:n], w_sb[:],
                                 start=True, stop=True)
                s3t = l3t.tile([128, wout], F32, tag="s3t" + str(wout))
                nc.scalar.copy(s3t[0:n, :], p3[0:n, 0:wout])
                dstv = dst_flat[lo * wout:hi * wout].rearrange(
                    "(p w) -> p w", p=n)
                nc.sync.dma_start(dstv, s3t[0:n, :])
                if b2_sink is not None:
                    nc.vector.tensor_mul(sqs[0:n, wout * j:wout * (j + 1)],
                                         s3t[0:n, :], s3t[0:n, :])
            if b2_sink is not None:
                b2_sink(sqs)

        # target stage0 -> baug planes 0..2 + b2 row 3
        def b2_t0(sqs):
            # chunks: 0,1=c0  2,3=c1  4,5=c2 ; per half-do [128,16]
            for half in range(2):
                a = sqs[:, 16 * half:16 * half + 16]
                bq = sqs[:, 32 + 16 * half:48 + 16 * half]
                cq = sqs[:, 64 + 16 * half:80 + 16 * half]
                t = sqp.tile([128, S0], F32, tag="b2t0")
                nc.vector.tensor_add(t[:], a, bq)
                nc.vector.tensor_add(t[:], t[:], cq)
                nc.vector.tensor_scalar_mul(t[:], t[:], -1.0)
                dstv = baug_d[0:1, 2048 * half:2048 * (half + 1)].rearrange(
                    "one (p w) -> (one p) w", p=128)
                nc.sync.dma_start(dstv, t[:])

        def b2_t1(sqs):
            # chunked by 64 rows -> one channel per chunk (column slices)
            t = sqp.tile([64, S1], F32, tag="b2t1")
            nc.vector.tensor_add(t[:], sqs[0:64, 0:8], sqs[0:64, 8:16])
            nc.vector.tensor_add(t[:], t[:], sqs[0:64, 16:24])
            nc.vector.tensor_scalar_mul(t[:], t[:], -1.0)
            dstv = b1aug_d[0:1, :].rearrange("one (p w) -> (one p) w", p=64)
            nc.sync.dma_start(dstv, t[:])

        baug_rows = baug_d.rearrange("a m -> (a m)")
        b1aug_rows = b1aug_d.rearrange("a m -> (a m)")
        l3(baug_rows[N0:4 * N0], y2t0, C * S0 * S0, S0, wwb0_sb, b2_t0)
        l3(b1aug_rows[N1:4 * N1], y2t1, C * S1 * S1, S1, wwb1_sb, b2_t1,
           crows=64)
        l3(aaug3_d, y2s0, C * S1 * S0, S0, wwa0_sb, None)
        l3(a1aug3_d, y2s1, C * 4 * S1, S1, wwa1_sb, None)

    if dbg == 1:
        return
    # ---------------- Phase S: distances + argmin ----------------
    args = ctx.enter_context(tc.tile_pool(name="sargs", bufs=1))
    idxp = ctx.enter_context(tc.tile_pool(name="idx", bufs=1))

    baug_sb = args.tile([4, N0], F32)
    nc.sync.dma_start(baug_sb[:], baug_d)
    aaug_sb = args.tile([4, N0 // 2], F32)
    nc.sync.dma_start(aaug_sb[1:4, :],
                      aaug3_d.rearrange("(c n) -> c n", c=3))
    nc.vector.memset(aaug_sb[0:1, :], 1.0)
    b1aug_sb = args.tile([4, N1], F32)
    nc.sync.dma_start(b1aug_sb[:], b1aug_d)
    a1aug_sb = args.tile([4, N1 // 2], F32)
    nc.sync.dma_start(a1aug_sb[1:4, :],
                      a1aug3_d.rearrange("(c n) -> c n", c=3))
    nc.vector.memset(a1aug_sb[0:1, :], 1.0)

    idx0_u = idxp.tile([128, 16], U32)
    idx1_u = idxp.tile([128, 2], U32)
    inm8 = idxp.tile([128, 8], F32)
    nc.vector.memset(inm8[:, 1:8], NEG)

    with tc.tile_pool(name="junk", bufs=2) as junkp, \
         tc.tile_pool(name="ssb", bufs=2) as ssbp, \
         tc.tile_pool(name="scal", bufs=2) as scal, \
         tc.tile_pool(name="psA", bufs=1, space="PSUM") as psa, \
         tc.tile_pool(name="psB", bufs=1, space="PSUM") as psb:
        for T in range(1 if dbg in (21, 22, 23) else 16):
            pa = psa.tile([128, 2048], F32, tag="pa")
            pb = psb.tile([128, 2048], F32, tag="pb")
            lhs = aaug_sb[:, 128 * T:128 * (T + 1)]
            for j in range(4):
                nc.tensor.matmul(pa[:, 512 * j:512 * (j + 1)], lhs,
                                 baug_sb[:, 512 * j:512 * (j + 1)],
                                 start=True, stop=True)
            for j in range(4):
                nc.tensor.matmul(pb[:, 512 * j:512 * (j + 1)], lhs,
                                 baug_sb[:, 2048 + 512 * j:2048 + 512 * (j + 1)],
                                 start=True, stop=True)
            s_sb = ssbp.tile([128, 4096], F32, tag="ssb")
            nc.scalar.copy(s_sb[:, 0:2048], pa[:])
            nc.scalar.copy(s_sb[:, 2048:4096], pb[:])
            if dbg == 21:
                nc.sync.dma_start(io['sdump'], s_sb[:])
                return
            t8 = scal.tile([128, 8], F32, tag="t8")
            nc.vector.max(t8[:], s_sb[:])
            i8 = scal.tile([128, 8], U32, tag="i8")
            nc.vector.max_index(i8[:], t8[:], s_sb[:])
            if dbg == 23:
                nc.sync.dma_start(io['sdump'][:, 0:8].bitcast(U32), i8[:])
                return
            nc.scalar.copy(idx0_u[:, T:T + 1], i8[:, 0:1])

    # stage 1
    with tc.tile_pool(name="junk1", bufs=2) as junkp, \
         tc.tile_pool(name="ssb1", bufs=2) as ssbp, \
         tc.tile_pool(name="scal1", bufs=2) as scal, \
         tc.tile_pool(name="ps1", bufs=2, space="PSUM") as ps1p:
        for T in range(2):
            p1 = ps1p.tile([128, 512], F32, tag="s1")
            nc.tensor.matmul(p1[:], a1aug_sb[:, 128 * T:128 * (T + 1)],
                             b1aug_sb[:], start=True, stop=True)
            s1_sb = ssbp.tile([128, 512], F32, tag="s1sb")
            nc.scalar.copy(s1_sb[:], p1[:])
            t81 = scal.tile([128, 8], F32, tag="t81")
            nc.vector.max(t81[:], s1_sb[:])
            i81 = scal.tile([128, 8], U32, tag="i81")
            nc.vector.max_index(i81[:], t81[:], s1_sb[:])
            nc.scalar.copy(idx1_u[:, T:T + 1], i81[:, 0:1])

    nc.sync.dma_start(io['idx0_out'], idx0_u[:])
    nc.sync.dma_start(io['idx1_out'], idx1_u[:])
    if dbg == 2:
        return
    # ---------------- Phase G: gather + cosine ----------------
    with tc.tile_pool(name="gath", bufs=1) as gp, \
         tc.tile_pool(name="cosw", bufs=2) as cw, \
         tc.tile_pool(name="psF", bufs=1, space="PSUM") as psf:
        # build wrapped int16 index tables (idx i at partition i%16, col i//16,
        # replicated across the 8 gpsimd cores) via a DRAM bounce
        I16 = mybir.dt.int16
        nc.sync.dma_start(idx0_d.rearrange("(t p) -> p t", p=128), idx0_u[:])
        nc.sync.dma_start(idx1_d.rearrange("(t p) -> p t", p=128), idx1_u[:])
        idxs0_32 = gp.tile([128, 128], U32)
        idxs1_32 = gp.tile([128, 16], U32)
        for g in range(8):
            nc.sync.dma_start(idxs0_32[16 * g:16 * (g + 1), :],
                              idx0_d.rearrange("(s r) -> r s", r=16))
            nc.sync.dma_start(idxs1_32[16 * g:16 * (g + 1), :],
                              idx1_d.rearrange("(s r) -> r s", r=16))
        idxs0_sb = gp.tile([128, 128], I16)
        nc.vector.tensor_copy(idxs0_sb[:], idxs0_32[:])
        idxs1_sb = gp.tile([128, 16], I16)
        nc.vector.tensor_copy(idxs1_sb[:], idxs1_32[:])

        if dbg == 30:
            nc.sync.dma_start(io['sdump'][:, 0:64].bitcast(I16),
                              idxs0_sb[:])
            return
        gd0p = gp.tile([128, 16, 2 * CD], F32)
        nc.gpsimd.dma_gather(
            out_ap=gd0p[:], in_ap=td0T, idxs_ap=idxs0_sb[:],
            num_idxs=N0 // 2, num_idxs_reg=N0 // 2, elem_size=2 * CD,
            single_packet=False)
        gd0 = gd0p[:, :, 0:CD]
        sd0 = gp.tile([128, 16, CD], F32)
        nc.sync.dma_start(sd0[:], sd0T.rearrange("(t p) c -> p t c", p=128))
        gd1p = gp.tile([128, 2, 2 * CD], F32)
        nc.gpsimd.dma_gather(
            out_ap=gd1p[:], in_ap=td1T, idxs_ap=idxs1_sb[:],
            num_idxs=N1 // 2, num_idxs_reg=N1 // 2, elem_size=2 * CD,
            single_packet=False)
        gd1 = gd1p[:, :, 0:CD]
        sd1 = gp.tile([128, 2, CD], F32)
        nc.sync.dma_start(sd1[:], sd1T.rearrange("(t p) c -> p t c", p=128))

        if dbg == 31:
            nc.sync.dma_start(io['sdump'][:, 0:512], gd0)
            return
        if dbg == 32:
            nc.sync.dma_start(io['sdump'][:, 0:512],
                              sd0[:].rearrange("p t c -> p (t c)"))
            return
        cs01 = gp.tile([128, 2], F32)
        ones_sb = gp.tile([128, 1], F32)
        nc.vector.memset(ones_sb[:], 1.0)

        for st, (gd, sd, nt) in enumerate([(gd0, sd0, 16), (gd1, sd1, 2)]):
            prod = cw.tile([128, nt, CD], F32, tag="prod" + str(st))
            num = cw.tile([128, nt], F32, tag="num" + str(st))
            nc.vector.tensor_mul(prod[:], sd[:], gd)
            nc.vector.reduce_sum(num[:], prod[:], axis=mybir.AxisListType.X)
            nc.vector.tensor_mul(prod[:], sd[:], sd[:])
            sn = cw.tile([128, nt], F32, tag="sn" + str(st))
            nc.vector.reduce_sum(sn[:], prod[:], axis=mybir.AxisListType.X)
            nc.vector.tensor_mul(prod[:], gd, gd)
            gn = cw.tile([128, nt], F32, tag="gn" + str(st))
            nc.vector.reduce_sum(gn[:], prod[:], axis=mybir.AxisListType.X)
            nc.scalar.activation(sn[:], sn[:], ACTF.Sqrt)
            nc.scalar.activation(gn[:], gn[:], ACTF.Sqrt)
            nc.vector.tensor_scalar_max(sn[:], sn[:], 1e-8)
            nc.vector.tensor_scalar_max(gn[:], gn[:], 1e-8)
            nc.vector.tensor_mul(sn[:], sn[:], gn[:])
            nc.vector.reciprocal(sn[:], sn[:])
            nc.vector.tensor_mul(num[:], num[:], sn[:])
            nc.vector.reduce_sum(cs01[:, st:st + 1], num[:],
                                 axis=mybir.AxisListType.X)

        pf = psf.tile([2, 1], F32)
        nc.tensor.matmul(pf[:], cs01[:], ones_sb[:], start=True, stop=True)
        of = gp.tile([2, 1], F32)
        nc.scalar.copy(of[:], pf[:])
        nc.sync.dma_start(out.rearrange("(a one) -> a one", one=1), of[:])


def _build_program(dbg=0):
    nc = bacc.Bacc("TRN2", target_bir_lowering=False, debug=False,
                   enable_asserts=True, num_devices=NCORES)
    io = {}
    io['_dbg'] = dbg

    def inp(name, shape):
        io[name] = nc.dram_tensor(name, list(shape), F32,
                                  kind="ExternalInput").ap()

    inp('ct', (C, D, D, D))
    inp('cs', (C, _SRC_DN, D, D))
    inp('wdt', (64, 24))
    inp('wds', (_SRC_DN, 12))
    inp('wh0', (64, S0))
    inp('wh1', (64, S1))
    inp('wwb0', (64, S0))
    inp('wwa0', (64, S0))
    inp('wwb1', (64, S1))
    inp('wwa1', (64, S1))
    inp('td0T', (N0, 2 * CD))
    inp('sd0T', (N0 // 2, CD))
    inp('td1T', (N1, 2 * CD))
    inp('sd1T', (N1 // 2, CD))
    inp('ident', (128, 128))
    io['out'] = nc.dram_tensor('out', [2], F32, kind="ExternalOutput").ap()
    io['idx0_out'] = nc.dram_tensor('idx0_out', [128, 16], U32,
                                    kind="ExternalOutput").ap()
    io['idx1_out'] = nc.dram_tensor('idx1_out', [128, 2], U32,
                                    kind="ExternalOutput").ap()
    io['sdump'] = nc.dram_tensor('sdump', [128, 4096], F32,
                                 kind="ExternalOutput").ap()

    def scratch(name, shape, dtype=F32):
        kw = {'kind': 'ExternalOutput'} if dbg >= 1 else {}
        io[name] = nc.dram_tensor(name, list(shape), dtype, **kw).ap()

    scratch('y1t0', (C, S0, D, D))
    scratch('y1t1', (C, S1, D, D))
    scratch('y1s0', (C, S1, D, D))
    scratch('y1s1', (C, 4, D, D))
    scratch('y2t0', (C, S0, S0, D))
    scratch('y2t1', (C, S1, S1, D))
    scratch('y2s0', (C, S1, S0, D))
    scratch('y2s1', (C, 4, S1, D))
    scratch('baug', (4, N0))
    scratch('b1aug', (4, N1))
    scratch('aaug3', (C * N0 // 2,))
    scratch('a1aug3', (C * N1 // 2,))
    scratch('idx0_d', (N0 // 2,), mybir.dt.uint32)
    scratch('idx1_d', (N1 // 2,), mybir.dt.uint32)

    with tile.TileContext(nc, trace_sim=False) as tc:
        _kern(tc, io)
    nc.compile()
    return nc


_CACHE = {}


def _program(dbg=0):
    key = ('nc', dbg)
    if key not in _CACHE:
        _CACHE[key] = _build_program(dbg)
    return _CACHE[key]


def _host_inputs(canonical_source, canonical_target, src_desc0, tgt_desc0,
                 src_desc1, tgt_desc1):
    w0 = _resize_weights(D, S0)   # [64,16]
    w1 = _resize_weights(D, S1)   # [64,8]
    wdt = np.concatenate([w0, w1], axis=1)               # [64,24]
    ident = np.eye(128, dtype=np.float32)
    in_maps = []
    for core in range(NCORES):
        b, h = divmod(core, 2)
        d0 = _SRC_D0[h]
        wds = np.concatenate([w0[d0:d0 + _SRC_DN, 8 * h:8 * h + 8],
                              w1[d0:d0 + _SRC_DN, 4 * h:4 * h + 4]], axis=1)
        m = {
            'ct': np.ascontiguousarray(canonical_target[b]),
            'cs': np.ascontiguousarray(canonical_source[b][:, d0:d0 + _SRC_DN]),
            'wdt': wdt, 'wds': np.ascontiguousarray(wds),
            'wh0': w0, 'wh1': w1,
            'wwb0': w0, 'wwa0': (2.0 * w0).astype(np.float32),
            'wwb1': w1, 'wwa1': (2.0 * w1).astype(np.float32),
            'td0T': np.ascontiguousarray(np.pad(
                tgt_desc0[b].reshape(CD, N0).T, ((0, 0), (0, CD)))),
            'sd0T': np.ascontiguousarray(
                src_desc0[b].reshape(CD, N0).T[h * 2048:(h + 1) * 2048]),
            'td1T': np.ascontiguousarray(np.pad(
                tgt_desc1[b].reshape(CD, N1).T, ((0, 0), (0, CD)))),
            'sd1T': np.ascontiguousarray(
                src_desc1[b].reshape(CD, N1).T[h * 256:(h + 1) * 256]),
            'ident': ident,
        }
        in_maps.append(m)
    return in_maps


def kernel(dbg=0, **inputs):
    inputs = {k: np.asarray(v, dtype=np.float32) for k, v in inputs.items()}
    nc = _program(dbg)
    in_maps = _host_inputs(**inputs)
    res = run_bass_kernel_spmd(nc, in_maps, list(range(NCORES)))
    _CACHE['last_res'] = res
    if dbg:
        return None
    parts = np.stack([res.results[c]['out'] for c in range(NCORES)])
    s0 = parts[:, 0].sum(dtype=np.float64)
    s1 = parts[:, 1].sum(dtype=np.float64)
    l0 = np.float32(1.0) - np.float32(s0 / (B * N0))
    l1 = np.float32(1.0) - np.float32(s1 / (B * N1))
    return np.float32((l0 + l1) / 2.0)



# revision 7
# speedup vs baseline: 1.4567x; 1.4567x over previous
"""Trainium2 Bass kernel for nn_DescriptorContrastiveLoss (optimized).

Contract: kernel(**inputs) takes FULL inputs (as produced by
reference.setup_inputs()) and returns the FULL scalar output.

Sharding: data-parallel over (batch, row-half): core c handles batch c//2,
row-half c%2.  Each core:
  - resizes canonical volumes in bf16 (trilinear + antialias, exact jax
    weights quantized to bf16) with three separable contractions (DRAM
    bounces between stages re-partition the data),
  - computes s[n,m] = 2<a_n,b_m> - |b_m|^2 via K=4 bf16 matmuls,
  - argmax per row via DVE running-max scan + is_lt count (exact
    first-occurrence semantics on the bf16 copy of s),
  - builds gather tables via PE transpose of the fp32 indices and gathers
    matched (host-prenormalized) target descriptors with dma_gather,
  - dots against host-prenormalized source descriptors and reduces.
Host combines the 8 partial sums into the final scalar loss.
"""
import sys

sys.path.insert(0, '/opt/trn_rl_repo')

import numpy as np
from contextlib import ExitStack

import concourse.bass as bass
import concourse.tile as tile
import concourse.bacc as bacc
import concourse.mybir as mybir
from concourse._compat import with_exitstack
from concourse.bass_utils import run_bass_kernel_spmd

F32 = mybir.dt.float32
BF16 = mybir.dt.bfloat16
I16 = mybir.dt.int16
ALU = mybir.AluOpType

B = 4
C = 3
D = 64          # input volume side
S0, S1 = 16, 8  # stage output sides
N0, N1 = S0 ** 3, S1 ** 3   # 4096, 512
CD = 32         # descriptor channels
NCORES = 8

# d-slice of the source volume needed per half (with filter support halo)
_SRC_D0 = {0: 0, 1: 28}
_SRC_DN = 36


def _resize_weights(in_size: int, out_size: int) -> np.ndarray:
    """fp32-faithful replica of jax.image resize weights (triangle kernel,
    antialias=True, translation=0).  Returns [in_size, out_size]."""
    scale = out_size / in_size
    inv_scale = np.float32(1.0 / scale)
    kernel_scale = np.float32(max(1.0 / scale, 1.0))
    sample_f = ((np.arange(out_size, dtype=np.float32) + np.float32(0.5))
                * inv_scale - np.float32(0.5))
    x = np.abs(sample_f[None, :]
               - np.arange(in_size, dtype=np.float32)[:, None]) / kernel_scale
    w = np.maximum(np.float32(0), np.float32(1) - x).astype(np.float32)
    tot = w.sum(axis=0, keepdims=True, dtype=np.float32)
    w = np.where(np.abs(tot) > 1000.0 * float(np.finfo(np.float32).eps),
                 w / np.where(tot != 0, tot, 1), 0).astype(np.float32)
    valid = (sample_f >= -0.5) & (sample_f <= in_size - 0.5)
    return np.where(valid[None, :], w, 0).astype(np.float32)


def _rho0(i):
    """gather slot -> local row, stage 0 (matches tableA/B layout)."""
    half = i // 1024
    i = i % 1024
    q, c = i % 16, i // 16
    return 1024 * half + 128 * (q % 8) + 64 * (q // 8) + c


def _rho1(i):
    """gather slot -> local row, stage 1."""
    q, c = i % 16, i // 16
    return 128 * (q % 2) + 16 * (q // 2) + c


@with_exitstack
def _kern(ctx: ExitStack, tc: tile.TileContext, io: dict):
    nc = tc.nc

    consts = ctx.enter_context(tc.tile_pool(name="consts", bufs=1))
    identb = consts.tile([128, 128], BF16)
    nc.sync.dma_start(identb[:], io['identb'])
    identf = consts.tile([128, 128], F32)
    nc.sync.dma_start(identf[:], io['identf'])

    rw = ctx.enter_context(tc.tile_pool(name="rw", bufs=1))
    wdt_sb = rw.tile([64, 24], BF16)
    nc.sync.dma_start(wdt_sb[:], io['wdt'])
    wds_sb = rw.tile([_SRC_DN, 12], BF16)
    nc.sync.dma_start(wds_sb[:], io['wds'])
    wh0_sb = rw.tile([64, S0], BF16)
    nc.sync.dma_start(wh0_sb[:], io['wh0'])
    wh1_sb = rw.tile([64, S1], BF16)
    nc.sync.dma_start(wh1_sb[:], io['wh1'])
    wwb0_sb = rw.tile([64, S0], BF16)
    nc.sync.dma_start(wwb0_sb[:], io['wwb0'])
    wwa0_sb = rw.tile([64, S0], BF16)
    nc.sync.dma_start(wwa0_sb[:], io['wwa0'])
    wwb1_sb = rw.tile([64, S1], BF16)
    nc.sync.dma_start(wwb1_sb[:], io['wwb1'])
    wwa1_sb = rw.tile([64, S1], BF16)
    nc.sync.dma_start(wwa1_sb[:], io['wwa1'])

    # augmented operands (SBUF), loaded from the DRAM staging buffers
    augp = ctx.enter_context(tc.tile_pool(name="aug", bufs=1))
    baug = augp.tile([4, N0], BF16)
    aaug = augp.tile([4, N0 // 2], BF16)
    b1aug = augp.tile([4, N1], BF16)
    a1aug = augp.tile([4, N1 // 2], BF16)
    nc.vector.memset(aaug[0:1, :], 1.0)
    nc.vector.memset(a1aug[0:1, :], 1.0)

    # descriptor tiles (loaded early, consumed by phase G)
    gathp = ctx.enter_context(tc.tile_pool(name="gath", bufs=1))
    sd0w = gathp.tile([128, S0, CD], F32)
    nc.sync.dma_start(sd0w[:], io['sd0w'])
    sd1w = gathp.tile([128, 2, CD], F32)
    nc.sync.dma_start(sd1w[:], io['sd1w'])
    gd0 = gathp.tile([128, S0, 2 * CD], F32)
    gd1 = gathp.tile([128, 2, 2 * CD], F32)
    table0 = gathp.tile([128, 128], I16)
    table1 = gathp.tile([128, 16], I16)
    idx0_f = gathp.tile([128, 16], F32)
    idx1_f = gathp.tile([128, 2], F32)
    cs01 = gathp.tile([128, 2], F32)
    ones_sb = gathp.tile([128, 1], F32)
    nc.vector.memset(ones_sb[:], 1.0)

    baug_d, aaug_d = io['baug_d'], io['aaug_d']
    b1aug_d, a1aug_d = io['b1aug_d'], io['a1aug_d']

    # ---------------- Phase R: resize ----------------
    with tc.tile_pool(name="l1in", bufs=1) as l1p, \
         tc.tile_pool(name="l1out", bufs=2) as l1o, \
         tc.tile_pool(name="y1", bufs=1) as y1p:

        ct_sb = l1p.tile([64, C * D * D], BF16)
        cs_sb = l1p.tile([_SRC_DN, C * D * D], BF16)
        for k in range(4):
            sl = slice(3072 * k, 3072 * (k + 1))
            nc.sync.dma_start(ct_sb[:, sl], io['ct'][:, sl])
            nc.sync.dma_start(cs_sb[:, sl], io['cs'][:, sl])

        # y1[h, c, do-slot, w]: slots 0:24 target (16 st0 + 8 st1),
        # 24:36 source (8 st0 + 4 st1)
        y1 = y1p.tile([64, C, 36, 64], BF16)
        y1t_d = io['y1t_d']   # [C, 24, 64, 64] (c, do, h, w)
        y1s_d = io['y1s_d']   # [C, 12, 64, 64]

        # L1: contract d. 6 chunks of 2048 cols (c, 32 h-rows); t rows 0:24,
        # s rows 32:44 of one PSUM tile.
        with tc.tile_pool(name="psl1", bufs=2, space="PSUM") as psl1:
            for k in range(6):
                p1 = psl1.tile([44, 2048], F32, tag="p1")
                for j in range(4):
                    sl = slice(2048 * k + 512 * j, 2048 * k + 512 * (j + 1))
                    nc.tensor.matmul(p1[0:24, 512 * j:512 * (j + 1)],
                                     wdt_sb[:], ct_sb[:, sl],
                                     start=True, stop=True)
                    nc.tensor.matmul(p1[32:44, 512 * j:512 * (j + 1)],
                                     wds_sb[:], cs_sb[:, sl],
                                     start=True, stop=True)
                s1t = l1o.tile([44, 2048], BF16, tag="s1t")
                nc.scalar.copy(s1t[:], p1[:])
                c, hlo = k // 2, 32 * (k % 2)
                sv = s1t[0:24, :].rearrange("p (h w) -> p h w", h=32)
                nc.sync.dma_start(y1t_d[c, :, hlo:hlo + 32, :], sv)
                sv2 = s1t[32:44, :].rearrange("p (h w) -> p h w", h=32)
                nc.sync.dma_start(y1s_d[c, :, hlo:hlo + 32, :], sv2)

        # reload re-partitioned (h in partitions), per c
        for c in range(C):
            nc.sync.dma_start(
                y1[:, c, 0:24, :],
                y1t_d[c].rearrange("do h w -> h do w"))
            nc.sync.dma_start(
                y1[:, c, 24:36, :],
                y1s_d[c].rearrange("do h w -> h do w"))

        # L2 (contract h) + L3 (contract w) + stage-1 distance pass S1.
        with tc.tile_pool(name="l2o", bufs=1) as l2o, \
             tc.tile_pool(name="l3in", bufs=2) as l3in, \
             tc.tile_pool(name="l3t", bufs=2) as l3t, \
             tc.tile_pool(name="sq", bufs=1) as sqp, \
             tc.tile_pool(name="s1sb", bufs=2) as s1sbp, \
             tc.tile_pool(name="scn", bufs=2) as scnp, \
             tc.tile_pool(name="psl2", bufs=1, space="PSUM") as psl2, \
             tc.tile_pool(name="pstr", bufs=1, space="PSUM") as pstr, \
             tc.tile_pool(name="psl3", bufs=2, space="PSUM") as psl3, \
             tc.tile_pool(name="pss1", bufs=1, space="PSUM") as pss1:

            y2t0_d = io['y2t0_d']   # [6, 128, 64] block=(c,do-par), row=(do%8,ho)
            y2t1_d = io['y2t1_d']   # [3, 64, 64]  block=c, row=(do,ho)
            y2s0_d = io['y2s0_d']   # [3, 128, 64] block=c, row=(do,ho)
            y2s1_d = io['y2s1_d']   # [96, 64]     row=(c,do,ho)

            for c in range(C):
                p2 = psl2.tile([80, 1024], F32, tag="p2")
                for j in range(2):
                    nc.tensor.matmul(p2[0:16, 512 * j:512 * (j + 1)],
                                     wh0_sb[:],
                                     y1[:, c, 0:16, :].rearrange(
                                         "h do w -> h (do w)")[:,
                                         512 * j:512 * (j + 1)],
                                     start=True, stop=True)
                nc.tensor.matmul(p2[32:40, 0:512], wh1_sb[:],
                                 y1[:, c, 16:24, :].rearrange(
                                     "h do w -> h (do w)"),
                                 start=True, stop=True)
                nc.tensor.matmul(p2[64:80, 0:512], wh0_sb[:],
                                 y1[:, c, 24:32, :].rearrange(
                                     "h do w -> h (do w)"),
                                 start=True, stop=True)
                p2b = psl2.tile([8, 256], F32, tag="p2b")
                nc.tensor.matmul(p2b[:], wh1_sb[:],
                                 y1[:, c, 32:36, :].rearrange(
                                     "h do w -> h (do w)"),
                                 start=True, stop=True)
                s2c = l2o.tile([80, 1024], BF16, tag="s2c")
                nc.scalar.copy(s2c[:], p2[:])
                s2b = l2o.tile([8, 256], BF16, tag="s2b")
                nc.scalar.copy(s2b[:], p2b[:])
                # scatter to y2 DRAM blocks (block row = (do,ho) raster)
                t0v = s2c[0:16, :].rearrange("ho (do w) -> ho do w", do=16)
                nc.sync.dma_start(
                    y2t0_d[2 * c].rearrange("(do ho) w -> ho do w", do=8),
                    t0v[:, 0:8, :])
                nc.sync.dma_start(
                    y2t0_d[2 * c + 1].rearrange("(do ho) w -> ho do w", do=8),
                    t0v[:, 8:16, :])
                t1v = s2c[32:40, 0:512].rearrange("ho (do w) -> ho do w", do=8)
                nc.sync.dma_start(
                    y2t1_d[c].rearrange("(do ho) w -> ho do w", do=8), t1v)
                s0v = s2c[64:80, 0:512].rearrange("ho (do w) -> ho do w", do=8)
                nc.sync.dma_start(
                    y2s0_d[c].rearrange("(do ho) w -> ho do w", do=8), s0v)
                s1v = s2b[:].rearrange("ho (do w) -> ho do w", do=4)
                nc.sync.dma_start(
                    y2s1_d[32 * c:32 * (c + 1), :].rearrange(
                        "(do ho) w -> ho do w", do=4), s1v)

            # ---- L3 target stage1 (3 chunks of 64 rows, one c each) + b2
            b1sq = sqp.tile([64, S1], F32, tag="b1sq")
            for c in range(C):
                lt = l3in.tile([64, 64], BF16, tag="lt1")
                nc.sync.dma_start(lt[:], y2t1_d[c])
                ptr = pstr.tile([64, 128], BF16, tag="ptr")
                nc.tensor.transpose(ptr[:, 0:64], lt[:], identb[0:64, 0:64])
                tr = l3t.tile([64, 64], BF16, tag="tr1")
                nc.scalar.copy(tr[:], ptr[:, 0:64])
                p3 = psl3.tile([128, S0], F32, tag="p3")
                nc.tensor.matmul(p3[0:64, 0:S1], tr[:], wwb1_sb[:],
                                 start=True, stop=True)
                s3t = l3t.tile([64, S1], BF16, tag="s3t1")
                nc.scalar.copy(s3t[:], p3[0:64, 0:S1])
                nc.sync.dma_start(
                    b1aug_d[1 + c:2 + c, :].rearrange(
                        "one (p w) -> (one p) w", p=64),
                    s3t[:])
                if c == 0:
                    nc.vector.tensor_mul(b1sq[:], s3t[:], s3t[:])
                else:
                    sq2 = sqp.tile([64, S1], F32, tag="b1sq2")
                    nc.vector.tensor_mul(sq2[:], s3t[:], s3t[:])
                    nc.vector.tensor_add(b1sq[:], b1sq[:], sq2[:])
            nc.vector.tensor_scalar_mul(b1sq[:], b1sq[:], -1.0)
            b1sqb = sqp.tile([64, S1], BF16, tag="b1sqb")
            nc.vector.tensor_copy(b1sqb[:], b1sq[:])
            nc.sync.dma_start(
                b1aug_d[0:1, :].rearrange("one (p w) -> (one p) w", p=64),
                b1sqb[:])
            nc.sync.dma_start(b1aug[:], b1aug_d)

            # ---- L3 source stage1 (1 chunk of 96 rows, 3 c's)
            lt96 = l3in.tile([96, 64], BF16, tag="lt96")
            nc.sync.dma_start(lt96[:], y2s1_d)
            ptr = pstr.tile([64, 128], BF16, tag="ptr")
            nc.tensor.transpose(ptr[:, 0:96], lt96[:], identb[0:96, 0:96])
            tr = l3t.tile([64, 96], BF16, tag="trs1")
            nc.scalar.copy(tr[:], ptr[:, 0:96])
            p3 = psl3.tile([128, S0], F32, tag="p3")
            nc.tensor.matmul(p3[0:96, 0:S1], tr[:], wwa1_sb[:],
                             start=True, stop=True)
            s3a1 = l3t.tile([96, S1], BF16, tag="s3a1")
            nc.scalar.copy(s3a1[:], p3[0:96, 0:S1])
            nc.sync.dma_start(
                a1aug_d[1:4, :].rearrange("c (p w) -> (c p) w", p=32),
                s3a1[:])
            nc.sync.dma_start(a1aug[1:4, :], a1aug_d[1:4, :])

            # ---- S1: stage-1 distances + scan argmax (overlaps L3 t0/s0)
            for T in range(2):
                ps1 = pss1.tile([128, 512], F32, tag="ps1")
                nc.tensor.matmul(ps1[:], a1aug[:, 128 * T:128 * (T + 1)],
                                 b1aug[:], start=True, stop=True)
                s1sb = s1sbp.tile([128, 512], BF16, tag="s1sb")
                nc.scalar.copy(s1sb[:], ps1[:])
                R = scnp.tile([128, 512], BF16, tag="R1")
                nc.vector.tensor_tensor_scan(R[:], s1sb[:], s1sb[:],
                                             -3.0e38, ALU.max, ALU.max)
                gmax = scnp.tile([128, 1], F32, tag="g1")
                nc.scalar.copy(gmax[:], R[:, 511:512])
                junk = scnp.tile([128, 512], BF16, tag="j1")
                cnt = scnp.tile([128, 1], F32, tag="c1")
                nc.vector.tensor_scalar(junk[:], R[:], gmax[:], 0.0,
                                        ALU.is_lt, ALU.add,
                                        accum_out=cnt[:])
                nc.scalar.copy(idx1_f[:, T:T + 1], cnt[:])

            # stage-1 gather table: transpose idx -> [2,128] -> i16 -> DRAM
            # DRAM layout = table layout: flat = g8*32 + t*16 + c
            ptr1 = pstr.tile([64, 128], F32, tag="ptr1f")
            nc.tensor.transpose(ptr1[0:2, :], idx1_f[:], identf[:])
            tr1c = l3t.tile([2, 128], I16, tag="tr1c")
            nc.vector.tensor_copy(tr1c[:], ptr1[0:2, :])
            nc.sync.dma_start(
                io['idx1_d'].rearrange("(g8 t c) -> t g8 c", t=2, c=16),
                tr1c[:].rearrange("t (g8 c) -> t g8 c", c=16))
            tb1v = io['idx1_d'].rearrange("(q c) -> q c", c=16)
            for g in range(8):
                nc.sync.dma_start(table1[16 * g:16 * (g + 1), :], tb1v)
            nc.gpsimd.dma_gather(
                out_ap=gd1[:], in_ap=io['td1T'], idxs_ap=table1[:],
                num_idxs=N1 // 2, num_idxs_reg=N1 // 2, elem_size=2 * CD,
                single_packet=False)

            # ---- L3 target stage0 (6 blocks) + b2
            b2sq = sqp.tile([128, S0, 2], F32, tag="b2sq")
            for j in range(6):
                c = j // 2
                par = j % 2
                lt = l3in.tile([128, 64], BF16, tag="lt0")
                nc.sync.dma_start(lt[:], y2t0_d[j])
                ptr = pstr.tile([64, 128], BF16, tag="ptr")
                nc.tensor.transpose(ptr[:], lt[:], identb[:])
                tr = l3t.tile([64, 128], BF16, tag="tr0")
                nc.scalar.copy(tr[:], ptr[:])
                p3 = psl3.tile([128, S0], F32, tag="p3")
                nc.tensor.matmul(p3[:], tr[:], wwb0_sb[:],
                                 start=True, stop=True)
                s3t = l3t.tile([128, S0], BF16, tag="s3t0")
                nc.scalar.copy(s3t[:], p3[:])
                nc.sync.dma_start(
                    baug_d[1 + c:2 + c, 2048 * par:2048 * (par + 1)].rearrange(
                        "one (p w) -> (one p) w", p=128),
                    s3t[:])
                if c == 0:
                    nc.vector.tensor_mul(b2sq[:, :, par], s3t[:], s3t[:])
                else:
                    sq2 = sqp.tile([128, S0], F32, tag="b2sq2")
                    nc.vector.tensor_mul(sq2[:], s3t[:], s3t[:])
                    nc.vector.tensor_add(b2sq[:, :, par], b2sq[:, :, par],
                                         sq2[:])
            for par in range(2):
                nc.vector.tensor_scalar_mul(b2sq[:, :, par], b2sq[:, :, par],
                                            -1.0)
                b2b = sqp.tile([128, S0], BF16, tag="b2b")
                nc.vector.tensor_copy(b2b[:], b2sq[:, :, par])
                nc.sync.dma_start(
                    baug_d[0:1, 2048 * par:2048 * (par + 1)].rearrange(
                        "one (p w) -> (one p) w", p=128),
                    b2b[:])
            nc.sync.dma_start(baug[:], baug_d)

            # ---- L3 source stage0 (3 blocks)
            for c in range(C):
                lt = l3in.tile([128, 64], BF16, tag="lt0")
                nc.sync.dma_start(lt[:], y2s0_d[c])
                ptr = pstr.tile([64, 128], BF16, tag="ptr")
                nc.tensor.transpose(ptr[:], lt[:], identb[:])
                tr = l3t.tile([64, 128], BF16, tag="trs0")
                nc.scalar.copy(tr[:], ptr[:])
                p3 = psl3.tile([128, S0], F32, tag="p3")
                nc.tensor.matmul(p3[:], tr[:], wwa0_sb[:],
                                 start=True, stop=True)
                s3a = l3t.tile([128, S0], BF16, tag="s3a0")
                nc.scalar.copy(s3a[:], p3[:])
                nc.sync.dma_start(
                    aaug_d[1 + c:2 + c, :].rearrange(
                        "one (p w) -> (one p) w", p=128),
                    s3a[:])
            nc.sync.dma_start(aaug[1:4, :], aaug_d[1:4, :])

    # ---------------- Phase S0: distances + scan argmax ----------------
    with tc.tile_pool(name="ssb", bufs=2) as ssbp, \
         tc.tile_pool(name="scn0", bufs=2) as scnp, \
         tc.tile_pool(name="psA", bufs=1, space="PSUM") as psa, \
         tc.tile_pool(name="psB", bufs=1, space="PSUM") as psb, \
         tc.tile_pool(name="pstr0", bufs=1, space="PSUM") as pstr0:

        def build_table0(half):
            """transpose idx cols -> DRAM (in table layout) -> table0."""
            ptrh = pstr0.tile([8, 128], F32, tag="ptrh")
            nc.tensor.transpose(ptrh[:], idx0_f[:, 8 * half:8 * (half + 1)],
                                identf[:])
            trc = scnp.tile([8, 128], I16, tag="trc" + str(half))
            nc.vector.tensor_copy(trc[:], ptrh[:])
            dv = io['idx0A_d'] if half == 0 else io['idx0B_d']
            # table layout: flat = g2*512 + t*64 + c  (q = g2*8 + t)
            nc.sync.dma_start(
                dv.rearrange("(g2 t c) -> t g2 c", t=8, c=64),
                trc[:].rearrange("t (g2 c) -> t g2 c", c=64))
            tbv = dv.rearrange("(q c) -> q c", c=64)
            for g in range(8):
                nc.sync.dma_start(
                    table0[16 * g:16 * (g + 1), 64 * half:64 * (half + 1)],
                    tbv)
            nc.gpsimd.dma_gather(
                out_ap=gd0[:, 8 * half:8 * (half + 1), :], in_ap=io['td0T'],
                idxs_ap=table0[:, 64 * half:64 * (half + 1)],
                num_idxs=N0 // 4, num_idxs_reg=N0 // 4, elem_size=2 * CD,
                single_packet=False)

        for T in range(16):
            lhs = aaug[:, 128 * T:128 * (T + 1)]
            s_sb = ssbp.tile([128, 4096], BF16, tag="ssb")
            for q in range(4):
                pool = psa if q % 2 == 0 else psb
                ph = pool.tile([128, 1024], F32, tag="p" + str(q % 2))
                for j in range(2):
                    nc.tensor.matmul(ph[:, 512 * j:512 * (j + 1)], lhs,
                                     baug[:, 1024 * q + 512 * j:
                                          1024 * q + 512 * (j + 1)],
                                     start=True, stop=True)
                nc.scalar.copy(s_sb[:, 1024 * q:1024 * (q + 1)], ph[:])
            R = scnp.tile([128, 4096], BF16, tag="R0")
            nc.vector.tensor_tensor_scan(R[:, 0:2048], s_sb[:, 0:2048],
                                         s_sb[:, 0:2048], -3.0e38,
                                         ALU.max, ALU.max)
            nc.vector.tensor_tensor_scan(R[:, 2048:4096], s_sb[:, 2048:4096],
                                         s_sb[:, 2048:4096], R[:, 2047:2048],
                                         ALU.max, ALU.max)
            gmax = scnp.tile([128, 1], F32, tag="g0")
            nc.scalar.copy(gmax[:], R[:, 4095:4096])
            junk = scnp.tile([128, 4096], BF16, tag="j0")
            cnt = scnp.tile([128, 2], F32, tag="c0")
            nc.vector.tensor_scalar(junk[:, 0:2048], R[:, 0:2048], gmax[:],
                                    0.0, ALU.is_lt, ALU.add,
                                    accum_out=cnt[:, 0:1])
            nc.vector.tensor_scalar(junk[:, 2048:4096], R[:, 2048:4096],
                                    gmax[:], 0.0, ALU.is_lt, ALU.add,
                                    accum_out=cnt[:, 1:2])
            nc.vector.tensor_add(idx0_f[:, T:T + 1], cnt[:, 0:1], cnt[:, 1:2])
            if T == 7:
                build_table0(0)
        build_table0(1)

        # ---------------- Phase G: cosine + reduce ----------------
        cw = scnp  # reuse pool for small cosine tiles
        prod = cw.tile([128, S0, CD], F32, tag="prod")
        nc.vector.tensor_mul(prod[:, 0:8, :], sd0w[:, 0:8, :],
                             gd0[:, 0:8, 0:CD])
        nc.vector.tensor_mul(prod[:, 8:16, :], sd0w[:, 8:16, :],
                             gd0[:, 8:16, 0:CD])
        cos0 = cw.tile([128, S0], F32, tag="cos0")
        nc.vector.reduce_sum(cos0[:], prod[:], axis=mybir.AxisListType.X)
        nc.vector.reduce_sum(cs01[:, 0:1], cos0[:], axis=mybir.AxisListType.X)
        prod1 = cw.tile([128, 2, CD], F32, tag="prod1")
        nc.vector.tensor_mul(prod1[:], sd1w[:], gd1[:, :, 0:CD])
        cos1 = cw.tile([128, 2], F32, tag="cos1")
        nc.vector.reduce_sum(cos1[:], prod1[:], axis=mybir.AxisListType.X)
        nc.vector.reduce_sum(cs01[:, 1:2], cos1[:], axis=mybir.AxisListType.X)

        pf = pstr0.tile([2, 1], F32, tag="pf")
        nc.tensor.matmul(pf[:], cs01[:], ones_sb[:], start=True, stop=True)
        of = cw.tile([2, 1], F32, tag="of")
        nc.scalar.copy(of[:], pf[:])
        nc.sync.dma_start(io['out'].rearrange("(a one) -> a one", one=1),
                          of[:])


def _build_program():
    nc = bacc.Bacc("TRN2", target_bir_lowering=False, debug=False,
                   enable_asserts=True, num_devices=NCORES)
    io = {}

    def inp(name, shape, dt=F32):
        io[name] = nc.dram_tensor(name, list(shape), dt,
                                  kind="ExternalInput").ap()

    inp('ct', (64, C * D * D), BF16)
    inp('cs', (_SRC_DN, C * D * D), BF16)
    inp('wdt', (64, 24), BF16)
    inp('wds', (_SRC_DN, 12), BF16)
    inp('wh0', (64, S0), BF16)
    inp('wh1', (64, S1), BF16)
    inp('wwb0', (64, S0), BF16)
    inp('wwa0', (64, S0), BF16)
    inp('wwb1', (64, S1), BF16)
    inp('wwa1', (64, S1), BF16)
    inp('td0T', (N0, 2 * CD))
    inp('sd0w', (128, S0, CD))
    inp('td1T', (N1, 2 * CD))
    inp('sd1w', (128, 2, CD))
    inp('identb', (128, 128), BF16)
    inp('identf', (128, 128), F32)
    io['out'] = nc.dram_tensor('out', [2], F32, kind="ExternalOutput").ap()

    def scratch(name, shape, dt=BF16):
        io[name] = nc.dram_tensor(name, list(shape), dt).ap()

    scratch('y1t_d', (C, 24, D, D))
    scratch('y1s_d', (C, 12, D, D))
    scratch('y2t0_d', (6, 128, 64))
    scratch('y2t1_d', (C, 64, 64))
    scratch('y2s0_d', (C, 128, 64))
    scratch('y2s1_d', (96, 64))
    scratch('baug_d', (4, N0))
    scratch('aaug_d', (4, N0 // 2))
    scratch('b1aug_d', (4, N1))
    scratch('a1aug_d', (4, N1 // 2))
    scratch('idx0A_d', (N0 // 4,), I16)
    scratch('idx0B_d', (N0 // 4,), I16)
    scratch('idx1_d', (N1 // 2,), I16)

    with tile.TileContext(nc, trace_sim=False) as tc:
        _kern(tc, io)
    nc.compile()
    return nc


_CACHE = {}


def _program():
    if 'nc' not in _CACHE:
        _CACHE['nc'] = _build_program()
    return _CACHE['nc']


def _bf16(x):
    x = np.ascontiguousarray(x, dtype=np.float32)
    u = x.view(np.uint32)
    r = ((u >> 16) & 1) + np.uint32(0x7fff)
    out = ((u + r) & np.uint32(0xffff0000)).view(np.float32)
    import ml_dtypes
    return out.astype(ml_dtypes.bfloat16)


def _normalize_rows(x):
    n = np.maximum(np.linalg.norm(x.astype(np.float32), axis=-1,
                                  keepdims=True).astype(np.float32),
                   np.float32(1e-8))
    return (x / n).astype(np.float32)


def _host_inputs(canonical_source, canonical_target, src_desc0, tgt_desc0,
                 src_desc1, tgt_desc1):
    w0 = _resize_weights(D, S0)   # [64,16]
    w1 = _resize_weights(D, S1)   # [64,8]
    wdt = np.concatenate([w0, w1], axis=1)               # [64,24]
    identb = _bf16(np.eye(128, dtype=np.float32))
    identf = np.eye(128, dtype=np.float32)

    rho0 = np.array([_rho0(i) for i in range(N0 // 2)])
    rho1 = np.array([_rho1(i) for i in range(N1 // 2)])
    # slot i lands at out[p, j] with i = j*128 + p
    perm0 = rho0.reshape(S0, 128).T    # [128 p, 16 j] -> local row
    perm1 = rho1.reshape(2, 128).T     # [128 p, 2 j]

    in_maps = []
    for core in range(NCORES):
        b, h = divmod(core, 2)
        d0 = _SRC_D0[h]
        wds = np.concatenate([w0[d0:d0 + _SRC_DN, 8 * h:8 * h + 8],
                              w1[d0:d0 + _SRC_DN, 4 * h:4 * h + 4]], axis=1)
        ctv = np.ascontiguousarray(
            canonical_target[b].transpose(1, 0, 2, 3).reshape(64, -1))
        csv = np.ascontiguousarray(
            canonical_source[b][:, d0:d0 + _SRC_DN].transpose(1, 0, 2, 3)
            .reshape(_SRC_DN, -1))
        td0n = _normalize_rows(tgt_desc0[b].reshape(CD, N0).T)
        td1n = _normalize_rows(tgt_desc1[b].reshape(CD, N1).T)
        sd0n = _normalize_rows(
            src_desc0[b].reshape(CD, N0).T[h * 2048:(h + 1) * 2048])
        sd1n = _normalize_rows(
            src_desc1[b].reshape(CD, N1).T[h * 256:(h + 1) * 256])
        m = {
            'ct': _bf16(ctv), 'cs': _bf16(csv),
            'wdt': _bf16(wdt), 'wds': _bf16(wds),
            'wh0': _bf16(w0), 'wh1': _bf16(w1),
            'wwb0': _bf16(w0), 'wwa0': _bf16(2.0 * w0),
            'wwb1': _bf16(w1), 'wwa1': _bf16(2.0 * w1),
            'td0T': np.ascontiguousarray(np.pad(td0n, ((0, 0), (0, CD)))),
            'sd0w': np.ascontiguousarray(sd0n[perm0]),
            'td1T': np.ascontiguousarray(np.pad(td1n, ((0, 0), (0, CD)))),
            'sd1w': np.ascontiguousarray(sd1n[perm1]),
            'identb': identb, 'identf': identf,
        }
        in_maps.append(m)
    return in_maps


def kernel(**inputs):
    inputs = {k: np.asarray(v, dtype=np.float32) for k, v in inputs.items()}
    nc = _program()
    in_maps = _host_inputs(**inputs)
    res = run_bass_kernel_spmd(nc, in_maps, list(range(NCORES)))
    _CACHE['last_res'] = res
    parts = np.stack([np.asarray(res.results[c]['out'])
                      for c in range(NCORES)])
    s0 = parts[:, 0].sum(dtype=np.float64)
    s1 = parts[:, 1].sum(dtype=np.float64)
    l0 = np.float32(1.0) - np.float32(s0 / (B * N0))
    l1 = np.float32(1.0) - np.float32(s1 / (B * N1))
    return np.float32((l0 + l1) / 2.0)


# revision 8
# speedup vs baseline: 1.9433x; 1.3340x over previous
"""Trainium2 Bass kernel for nn_DescriptorContrastiveLoss (optimized).

Contract: kernel(**inputs) takes FULL inputs (as produced by
reference.setup_inputs()) and returns the FULL scalar output.

Sharding: data-parallel over (batch, row-half): core c handles batch c//2,
row-half c%2.  Each core:
  - resizes canonical volumes in bf16 (trilinear + antialias, exact jax
    weights quantized to bf16) with three separable contractions (DRAM
    bounces between stages re-partition the data),
  - computes s[n,m] = 2<a_n,b_m> - |b_m|^2 via K=4 bf16 matmuls,
  - argmax per row via DVE running-max scan + is_lt count (exact
    first-occurrence semantics on the bf16 copy of s),
  - builds gather tables via PE transpose of the fp32 indices and gathers
    matched (host-prenormalized) target descriptors with dma_gather,
  - dots against host-prenormalized source descriptors and reduces.
Host combines the 8 partial sums into the final scalar loss.
"""
import sys

sys.path.insert(0, '/opt/trn_rl_repo')

import numpy as np
from contextlib import ExitStack

import concourse.bass as bass
import concourse.tile as tile
import concourse.bacc as bacc
import concourse.mybir as mybir
from concourse._compat import with_exitstack
from concourse.bass_utils import run_bass_kernel_spmd

F32 = mybir.dt.float32
BF16 = mybir.dt.bfloat16
I16 = mybir.dt.int16
ALU = mybir.AluOpType

B = 4
C = 3
D = 64          # input volume side
S0, S1 = 16, 8  # stage output sides
N0, N1 = S0 ** 3, S1 ** 3   # 4096, 512
CD = 32         # descriptor channels
NCORES = 8

# d-slice of the source volume needed per half (with filter support halo)
_SRC_D0 = {0: 0, 1: 28}
_SRC_DN = 36


def _resize_weights(in_size: int, out_size: int) -> np.ndarray:
    """fp32-faithful replica of jax.image resize weights (triangle kernel,
    antialias=True, translation=0).  Returns [in_size, out_size]."""
    scale = out_size / in_size
    inv_scale = np.float32(1.0 / scale)
    kernel_scale = np.float32(max(1.0 / scale, 1.0))
    sample_f = ((np.arange(out_size, dtype=np.float32) + np.float32(0.5))
                * inv_scale - np.float32(0.5))
    x = np.abs(sample_f[None, :]
               - np.arange(in_size, dtype=np.float32)[:, None]) / kernel_scale
    w = np.maximum(np.float32(0), np.float32(1) - x).astype(np.float32)
    tot = w.sum(axis=0, keepdims=True, dtype=np.float32)
    w = np.where(np.abs(tot) > 1000.0 * float(np.finfo(np.float32).eps),
                 w / np.where(tot != 0, tot, 1), 0).astype(np.float32)
    valid = (sample_f >= -0.5) & (sample_f <= in_size - 0.5)
    return np.where(valid[None, :], w, 0).astype(np.float32)


def _rho0(i):
    """gather slot -> local row, stage 0 (matches tableA/B layout)."""
    half = i // 1024
    i = i % 1024
    q, c = i % 16, i // 16
    return 1024 * half + 128 * (q % 8) + 64 * (q // 8) + c


def _rho1(i):
    """gather slot -> local row, stage 1."""
    q, c = i % 16, i // 16
    return 128 * (q % 2) + 16 * (q // 2) + c


@with_exitstack
def _kern(ctx: ExitStack, tc: tile.TileContext, io: dict):
    nc = tc.nc

    consts = ctx.enter_context(tc.tile_pool(name="consts", bufs=1))
    identb = consts.tile([128, 128], BF16)
    nc.sync.dma_start(identb[:], io['identb'])
    identf = consts.tile([128, 128], F32)
    nc.sync.dma_start(identf[:], io['identf'])

    rw = ctx.enter_context(tc.tile_pool(name="rw", bufs=1))
    wdt_sb = rw.tile([64, 24], BF16)
    nc.sync.dma_start(wdt_sb[:], io['wdt'])
    wds_sb = rw.tile([_SRC_DN, 12], BF16)
    nc.sync.dma_start(wds_sb[:], io['wds'])
    wh0_sb = rw.tile([64, S0], BF16)
    nc.sync.dma_start(wh0_sb[:], io['wh0'])
    wh1_sb = rw.tile([64, S1], BF16)
    nc.sync.dma_start(wh1_sb[:], io['wh1'])
    wwb0_sb = rw.tile([64, S0], BF16)
    nc.sync.dma_start(wwb0_sb[:], io['wwb0'])
    wwa0_sb = rw.tile([64, S0], BF16)
    nc.sync.dma_start(wwa0_sb[:], io['wwa0'])
    wwb1_sb = rw.tile([64, S1], BF16)
    nc.sync.dma_start(wwb1_sb[:], io['wwb1'])
    wwa1_sb = rw.tile([64, S1], BF16)
    nc.sync.dma_start(wwa1_sb[:], io['wwa1'])

    # augmented operands (SBUF), loaded from the DRAM staging buffers
    augp = ctx.enter_context(tc.tile_pool(name="aug", bufs=1))
    baug = augp.tile([4, N0], BF16)
    aaug = augp.tile([4, N0 // 2], BF16)
    b1aug = augp.tile([4, N1], BF16)
    a1aug = augp.tile([4, N1 // 2], BF16)
    nc.vector.memset(aaug[0:1, :], 1.0)
    nc.vector.memset(a1aug[0:1, :], 1.0)

    # descriptor tiles (loaded early, consumed by phase G)
    gathp = ctx.enter_context(tc.tile_pool(name="gath", bufs=1))
    sd0w = gathp.tile([128, S0, CD], F32)
    nc.sync.dma_start(sd0w[:], io['sd0w'])
    sd1w = gathp.tile([128, 2, CD], F32)
    nc.sync.dma_start(sd1w[:], io['sd1w'])
    gd0 = gathp.tile([128, S0, 2 * CD], F32)
    gd1 = gathp.tile([128, 2, 2 * CD], F32)
    table0 = gathp.tile([128, 128], I16)
    table1 = gathp.tile([128, 16], I16)
    idx0_f = gathp.tile([128, 16], F32)
    idx1_f = gathp.tile([128, 2], F32)
    cs01 = gathp.tile([128, 2], F32)
    ones_sb = gathp.tile([128, 1], F32)
    nc.vector.memset(ones_sb[:], 1.0)

    baug_d, aaug_d = io['baug_d'], io['aaug_d']
    b1aug_d, a1aug_d = io['b1aug_d'], io['a1aug_d']

    # ---------------- Phase R: resize ----------------
    with tc.tile_pool(name="l1in", bufs=1) as l1p, \
         tc.tile_pool(name="l1out", bufs=2) as l1o, \
         tc.tile_pool(name="y1", bufs=1) as y1p:

        ct_sb = l1p.tile([64, C * D * D], BF16)
        cs_sb = l1p.tile([_SRC_DN, C * D * D], BF16)
        for k in range(4):
            sl = slice(3072 * k, 3072 * (k + 1))
            nc.sync.dma_start(ct_sb[:, sl], io['ct'][:, sl])
            nc.sync.dma_start(cs_sb[:, sl], io['cs'][:, sl])

        # y1[h, c, do-slot, w]: slots 0:24 target (16 st0 + 8 st1),
        # 24:36 source (8 st0 + 4 st1)
        y1 = y1p.tile([64, C, 36, 64], BF16)
        y1t_d = io['y1t_d']   # [C, 24, 64, 64] (c, do, h, w)
        y1s_d = io['y1s_d']   # [C, 12, 64, 64]

        # L1: contract d. 6 chunks of 2048 cols (c, 32 h-rows); t rows 0:24,
        # s rows 32:44 of one PSUM tile.
        with tc.tile_pool(name="psl1", bufs=2, space="PSUM") as psl1:
            for k in range(6):
                p1 = psl1.tile([44, 2048], F32, tag="p1")
                for j in range(4):
                    sl = slice(2048 * k + 512 * j, 2048 * k + 512 * (j + 1))
                    nc.tensor.matmul(p1[0:24, 512 * j:512 * (j + 1)],
                                     wdt_sb[:], ct_sb[:, sl],
                                     start=True, stop=True)
                    nc.tensor.matmul(p1[32:44, 512 * j:512 * (j + 1)],
                                     wds_sb[:], cs_sb[:, sl],
                                     start=True, stop=True)
                s1t = l1o.tile([44, 2048], BF16, tag="s1t")
                nc.scalar.copy(s1t[:], p1[:])
                c, hlo = k // 2, 32 * (k % 2)
                sv = s1t[0:24, :].rearrange("p (h w) -> p h w", h=32)
                nc.sync.dma_start(y1t_d[c, :, hlo:hlo + 32, :], sv)
                sv2 = s1t[32:44, :].rearrange("p (h w) -> p h w", h=32)
                nc.sync.dma_start(y1s_d[c, :, hlo:hlo + 32, :], sv2)

        # reload re-partitioned (h in partitions), per c
        for c in range(C):
            nc.sync.dma_start(
                y1[:, c, 0:24, :],
                y1t_d[c].rearrange("do h w -> h do w"))
            nc.sync.dma_start(
                y1[:, c, 24:36, :],
                y1s_d[c].rearrange("do h w -> h do w"))

        # L2 (contract h) + L3 (contract w) + stage-1 distance pass S1.
        with tc.tile_pool(name="l2o", bufs=1) as l2o, \
             tc.tile_pool(name="l3in", bufs=2) as l3in, \
             tc.tile_pool(name="l3t", bufs=2) as l3t, \
             tc.tile_pool(name="sq", bufs=1) as sqp, \
             tc.tile_pool(name="s1sb", bufs=2) as s1sbp, \
             tc.tile_pool(name="scn", bufs=2) as scnp, \
             tc.tile_pool(name="psl2", bufs=1, space="PSUM") as psl2, \
             tc.tile_pool(name="pstr", bufs=1, space="PSUM") as pstr, \
             tc.tile_pool(name="psl3", bufs=2, space="PSUM") as psl3, \
             tc.tile_pool(name="pss1", bufs=1, space="PSUM") as pss1:

            y2t0_d = io['y2t0_d']   # [6, 128, 64] block=(c,do-par), row=(do%8,ho)
            y2t1_d = io['y2t1_d']   # [3, 64, 64]  block=c, row=(do,ho)
            y2s0_d = io['y2s0_d']   # [3, 128, 64] block=c, row=(do,ho)
            y2s1_d = io['y2s1_d']   # [96, 64]     row=(c,do,ho)

            for c in range(C):
                p2 = psl2.tile([80, 1024], F32, tag="p2")
                for j in range(2):
                    nc.tensor.matmul(p2[0:16, 512 * j:512 * (j + 1)],
                                     wh0_sb[:],
                                     y1[:, c, 0:16, :].rearrange(
                                         "h do w -> h (do w)")[:,
                                         512 * j:512 * (j + 1)],
                                     start=True, stop=True)
                nc.tensor.matmul(p2[32:40, 0:512], wh1_sb[:],
                                 y1[:, c, 16:24, :].rearrange(
                                     "h do w -> h (do w)"),
                                 start=True, stop=True)
                nc.tensor.matmul(p2[64:80, 0:512], wh0_sb[:],
                                 y1[:, c, 24:32, :].rearrange(
                                     "h do w -> h (do w)"),
                                 start=True, stop=True)
                p2b = psl2.tile([8, 256], F32, tag="p2b")
                nc.tensor.matmul(p2b[:], wh1_sb[:],
                                 y1[:, c, 32:36, :].rearrange(
                                     "h do w -> h (do w)"),
                                 start=True, stop=True)
                s2c = l2o.tile([80, 1024], BF16, tag="s2c")
                nc.scalar.copy(s2c[:], p2[:])
                s2b = l2o.tile([8, 256], BF16, tag="s2b")
                nc.scalar.copy(s2b[:], p2b[:])
                # scatter to y2 DRAM blocks (block row = (do,ho) raster)
                t0v = s2c[0:16, :].rearrange("ho (do w) -> ho do w", do=16)
                nc.sync.dma_start(
                    y2t0_d[2 * c].rearrange("(do ho) w -> ho do w", do=8),
                    t0v[:, 0:8, :])
                nc.sync.dma_start(
                    y2t0_d[2 * c + 1].rearrange("(do ho) w -> ho do w", do=8),
                    t0v[:, 8:16, :])
                t1v = s2c[32:40, 0:512].rearrange("ho (do w) -> ho do w", do=8)
                nc.sync.dma_start(
                    y2t1_d[c].rearrange("(do ho) w -> ho do w", do=8), t1v)
                s0v = s2c[64:80, 0:512].rearrange("ho (do w) -> ho do w", do=8)
                nc.sync.dma_start(
                    y2s0_d[c].rearrange("(do ho) w -> ho do w", do=8), s0v)
                s1v = s2b[:].rearrange("ho (do w) -> ho do w", do=4)
                nc.sync.dma_start(
                    y2s1_d[32 * c:32 * (c + 1), :].rearrange(
                        "(do ho) w -> ho do w", do=4), s1v)

            # ---- L3 target stage1 (3 chunks of 64 rows, one c each) + b2
            b1sq = sqp.tile([64, S1], F32, tag="b1sq")
            for c in range(C):
                lt = l3in.tile([64, 64], BF16, tag="lt1")
                nc.sync.dma_start(lt[:], y2t1_d[c])
                ptr = pstr.tile([64, 128], BF16, tag="ptr")
                nc.tensor.transpose(ptr[:, 0:64], lt[:], identb[0:64, 0:64])
                tr = l3t.tile([64, 64], BF16, tag="tr1")
                nc.scalar.copy(tr[:], ptr[:, 0:64])
                p3 = psl3.tile([128, S0], F32, tag="p3")
                nc.tensor.matmul(p3[0:64, 0:S1], tr[:], wwb1_sb[:],
                                 start=True, stop=True)
                s3t = l3t.tile([64, S1], BF16, tag="s3t1")
                nc.scalar.copy(s3t[:], p3[0:64, 0:S1])
                nc.sync.dma_start(
                    b1aug_d[1 + c:2 + c, :].rearrange(
                        "one (p w) -> (one p) w", p=64),
                    s3t[:])
                if c == 0:
                    nc.vector.tensor_mul(b1sq[:], s3t[:], s3t[:])
                else:
                    sq2 = sqp.tile([64, S1], F32, tag="b1sq2")
                    nc.vector.tensor_mul(sq2[:], s3t[:], s3t[:])
                    nc.vector.tensor_add(b1sq[:], b1sq[:], sq2[:])
            nc.vector.tensor_scalar_mul(b1sq[:], b1sq[:], -1.0)
            b1sqb = sqp.tile([64, S1], BF16, tag="b1sqb")
            nc.vector.tensor_copy(b1sqb[:], b1sq[:])
            nc.sync.dma_start(
                b1aug_d[0:1, :].rearrange("one (p w) -> (one p) w", p=64),
                b1sqb[:])
            nc.sync.dma_start(b1aug[:], b1aug_d)

            # ---- L3 source stage1 (1 chunk of 96 rows, 3 c's)
            lt96 = l3in.tile([96, 64], BF16, tag="lt96")
            nc.sync.dma_start(lt96[:], y2s1_d)
            ptr = pstr.tile([64, 128], BF16, tag="ptr")
            nc.tensor.transpose(ptr[:, 0:96], lt96[:], identb[0:96, 0:96])
            tr = l3t.tile([64, 96], BF16, tag="trs1")
            nc.scalar.copy(tr[:], ptr[:, 0:96])
            p3 = psl3.tile([128, S0], F32, tag="p3")
            nc.tensor.matmul(p3[0:96, 0:S1], tr[:], wwa1_sb[:],
                             start=True, stop=True)
            s3a1 = l3t.tile([96, S1], BF16, tag="s3a1")
            nc.scalar.copy(s3a1[:], p3[0:96, 0:S1])
            nc.sync.dma_start(
                a1aug_d[1:4, :].rearrange("c (p w) -> (c p) w", p=32),
                s3a1[:])
            nc.sync.dma_start(a1aug[1:4, :], a1aug_d[1:4, :])

            # ---- S1: stage-1 distances + scan argmax (overlaps L3 t0/s0)
            for T in range(2):
                ps1 = pss1.tile([128, 512], F32, tag="ps1")
                nc.tensor.matmul(ps1[:], a1aug[:, 128 * T:128 * (T + 1)],
                                 b1aug[:], start=True, stop=True)
                s1sb = s1sbp.tile([128, 512], BF16, tag="s1sb")
                nc.scalar.copy(s1sb[:], ps1[:])
                t81 = scnp.tile([128, 8], BF16, tag="t81")
                nc.vector.max(t81[:], s1sb[:])
                i81 = scnp.tile([128, 8], mybir.dt.uint32, tag="i81")
                nc.vector.max_index(i81[:], t81[:], s1sb[:])
                nc.vector.tensor_copy(idx1_f[:, T:T + 1], i81[:, 0:1])

            # stage-1 gather table: transpose idx -> [2,128] -> i16 -> DRAM
            # DRAM layout = table layout: flat = g8*32 + t*16 + c
            ptr1 = pstr.tile([64, 128], F32, tag="ptr1f")
            nc.tensor.transpose(ptr1[0:2, :], idx1_f[:], identf[:])
            tr1c = l3t.tile([2, 128], I16, tag="tr1c")
            nc.vector.tensor_copy(tr1c[:], ptr1[0:2, :])
            nc.sync.dma_start(
                io['idx1_d'].rearrange("(g8 t c) -> t g8 c", t=2, c=16),
                tr1c[:].rearrange("t (g8 c) -> t g8 c", c=16))
            tb1v = io['idx1_d'].rearrange("(q c) -> q c", c=16)
            for g in range(8):
                nc.sync.dma_start(table1[16 * g:16 * (g + 1), :], tb1v)
            nc.gpsimd.dma_gather(
                out_ap=gd1[:], in_ap=io['td1T'], idxs_ap=table1[:],
                num_idxs=N1 // 2, num_idxs_reg=N1 // 2, elem_size=2 * CD,
                single_packet=False)

            # ---- L3 target stage0 (6 blocks) + b2
            b2sq = sqp.tile([128, S0, 2], F32, tag="b2sq")
            for j in range(6):
                c = j // 2
                par = j % 2
                lt = l3in.tile([128, 64], BF16, tag="lt0")
                nc.sync.dma_start(lt[:], y2t0_d[j])
                ptr = pstr.tile([64, 128], BF16, tag="ptr")
                nc.tensor.transpose(ptr[:], lt[:], identb[:])
                tr = l3t.tile([64, 128], BF16, tag="tr0")
                nc.scalar.copy(tr[:], ptr[:])
                p3 = psl3.tile([128, S0], F32, tag="p3")
                nc.tensor.matmul(p3[:], tr[:], wwb0_sb[:],
                                 start=True, stop=True)
                s3t = l3t.tile([128, S0], BF16, tag="s3t0")
                nc.scalar.copy(s3t[:], p3[:])
                nc.sync.dma_start(
                    baug_d[1 + c:2 + c, 2048 * par:2048 * (par + 1)].rearrange(
                        "one (p w) -> (one p) w", p=128),
                    s3t[:])
                if c == 0:
                    nc.vector.tensor_mul(b2sq[:, :, par], s3t[:], s3t[:])
                else:
                    sq2 = sqp.tile([128, S0], F32, tag="b2sq2")
                    nc.vector.tensor_mul(sq2[:], s3t[:], s3t[:])
                    nc.vector.tensor_add(b2sq[:, :, par], b2sq[:, :, par],
                                         sq2[:])
            for par in range(2):
                nc.vector.tensor_scalar_mul(b2sq[:, :, par], b2sq[:, :, par],
                                            -1.0)
                b2b = sqp.tile([128, S0], BF16, tag="b2b")
                nc.vector.tensor_copy(b2b[:], b2sq[:, :, par])
                nc.sync.dma_start(
                    baug_d[0:1, 2048 * par:2048 * (par + 1)].rearrange(
                        "one (p w) -> (one p) w", p=128),
                    b2b[:])
            nc.sync.dma_start(baug[:], baug_d)

            # ---- L3 source stage0 (3 blocks)
            for c in range(C):
                lt = l3in.tile([128, 64], BF16, tag="lt0")
                nc.sync.dma_start(lt[:], y2s0_d[c])
                ptr = pstr.tile([64, 128], BF16, tag="ptr")
                nc.tensor.transpose(ptr[:], lt[:], identb[:])
                tr = l3t.tile([64, 128], BF16, tag="trs0")
                nc.scalar.copy(tr[:], ptr[:])
                p3 = psl3.tile([128, S0], F32, tag="p3")
                nc.tensor.matmul(p3[:], tr[:], wwa0_sb[:],
                                 start=True, stop=True)
                s3a = l3t.tile([128, S0], BF16, tag="s3a0")
                nc.scalar.copy(s3a[:], p3[:])
                nc.sync.dma_start(
                    aaug_d[1 + c:2 + c, :].rearrange(
                        "one (p w) -> (one p) w", p=128),
                    s3a[:])
            nc.sync.dma_start(aaug[1:4, :], aaug_d[1:4, :])

    # ---------------- Phase S0: distances + scan argmax ----------------
    with tc.tile_pool(name="ssb", bufs=2) as ssbp, \
         tc.tile_pool(name="scn0", bufs=2) as scnp, \
         tc.tile_pool(name="psA", bufs=1, space="PSUM") as psa, \
         tc.tile_pool(name="psB", bufs=1, space="PSUM") as psb, \
         tc.tile_pool(name="pstr0", bufs=1, space="PSUM") as pstr0:

        def build_table0(half):
            """transpose idx cols -> DRAM (in table layout) -> table0."""
            ptrh = pstr0.tile([8, 128], F32, tag="ptrh")
            nc.tensor.transpose(ptrh[:], idx0_f[:, 8 * half:8 * (half + 1)],
                                identf[:])
            trc = scnp.tile([8, 128], I16, tag="trc" + str(half))
            nc.vector.tensor_copy(trc[:], ptrh[:])
            dv = io['idx0A_d'] if half == 0 else io['idx0B_d']
            # table layout: flat = g2*512 + t*64 + c  (q = g2*8 + t)
            nc.sync.dma_start(
                dv.rearrange("(g2 t c) -> t g2 c", t=8, c=64),
                trc[:].rearrange("t (g2 c) -> t g2 c", c=64))
            tbv = dv.rearrange("(q c) -> q c", c=64)
            for g in range(8):
                nc.sync.dma_start(
                    table0[16 * g:16 * (g + 1), 64 * half:64 * (half + 1)],
                    tbv)
            nc.gpsimd.dma_gather(
                out_ap=gd0[:, 8 * half:8 * (half + 1), :], in_ap=io['td0T'],
                idxs_ap=table0[:, 64 * half:64 * (half + 1)],
                num_idxs=N0 // 4, num_idxs_reg=N0 // 4, elem_size=2 * CD,
                single_packet=False)

        for T in range(16):
            lhs = aaug[:, 128 * T:128 * (T + 1)]
            s_sb = ssbp.tile([128, 4096], BF16, tag="ssb")
            for q in range(4):
                pool = psa if q % 2 == 0 else psb
                ph = pool.tile([128, 1024], F32, tag="p" + str(q % 2))
                for j in range(2):
                    nc.tensor.matmul(ph[:, 512 * j:512 * (j + 1)], lhs,
                                     baug[:, 1024 * q + 512 * j:
                                          1024 * q + 512 * (j + 1)],
                                     start=True, stop=True)
                nc.scalar.copy(s_sb[:, 1024 * q:1024 * (q + 1)], ph[:])
            sv = s_sb[:].rearrange("p (blk r) -> p blk r", blk=32)
            f = scnp.tile([128, 32, 64], BF16, tag="f0")
            nc.vector.tensor_tensor(f[:], sv[:, :, 0:64], sv[:, :, 64:128],
                                    ALU.max)
            nc.vector.tensor_tensor(f[:, :, 0:32], f[:, :, 0:32],
                                    f[:, :, 32:64], ALU.max)
            nc.vector.tensor_tensor(f[:, :, 0:16], f[:, :, 0:16],
                                    f[:, :, 16:32], ALU.max)
            nc.vector.tensor_tensor(f[:, :, 0:8], f[:, :, 0:8],
                                    f[:, :, 8:16], ALU.max)
            t8 = scnp.tile([128, 8], BF16, tag="t8")
            nc.vector.max(t8[:], f[:, :, 0:8])
            i8 = scnp.tile([128, 8], mybir.dt.uint32, tag="i8")
            nc.vector.max_index(i8[:], t8[:], s_sb[:])
            nc.vector.tensor_copy(idx0_f[:, T:T + 1], i8[:, 0:1])
            if T == 7:
                build_table0(0)
        build_table0(1)

        # ---------------- Phase G: cosine + reduce ----------------
        cw = scnp  # reuse pool for small cosine tiles
        prod = cw.tile([128, S0, CD], F32, tag="prod")
        nc.vector.tensor_mul(prod[:, 0:8, :], sd0w[:, 0:8, :],
                             gd0[:, 0:8, 0:CD])
        nc.vector.tensor_mul(prod[:, 8:16, :], sd0w[:, 8:16, :],
                             gd0[:, 8:16, 0:CD])
        cos0 = cw.tile([128, S0], F32, tag="cos0")
        nc.vector.reduce_sum(cos0[:], prod[:], axis=mybir.AxisListType.X)
        nc.vector.reduce_sum(cs01[:, 0:1], cos0[:], axis=mybir.AxisListType.X)
        prod1 = cw.tile([128, 2, CD], F32, tag="prod1")
        nc.vector.tensor_mul(prod1[:], sd1w[:], gd1[:, :, 0:CD])
        cos1 = cw.tile([128, 2], F32, tag="cos1")
        nc.vector.reduce_sum(cos1[:], prod1[:], axis=mybir.AxisListType.X)
        nc.vector.reduce_sum(cs01[:, 1:2], cos1[:], axis=mybir.AxisListType.X)

        pf = pstr0.tile([2, 1], F32, tag="pf")
        nc.tensor.matmul(pf[:], cs01[:], ones_sb[:], start=True, stop=True)
        of = cw.tile([2, 1], F32, tag="of")
        nc.scalar.copy(of[:], pf[:])
        nc.sync.dma_start(io['out'].rearrange("(a one) -> a one", one=1),
                          of[:])


def _build_program():
    nc = bacc.Bacc("TRN2", target_bir_lowering=False, debug=False,
                   enable_asserts=True, num_devices=NCORES)
    io = {}

    def inp(name, shape, dt=F32):
        io[name] = nc.dram_tensor(name, list(shape), dt,
                                  kind="ExternalInput").ap()

    inp('ct', (64, C * D * D), BF16)
    inp('cs', (_SRC_DN, C * D * D), BF16)
    inp('wdt', (64, 24), BF16)
    inp('wds', (_SRC_DN, 12), BF16)
    inp('wh0', (64, S0), BF16)
    inp('wh1', (64, S1), BF16)
    inp('wwb0', (64, S0), BF16)
    inp('wwa0', (64, S0), BF16)
    inp('wwb1', (64, S1), BF16)
    inp('wwa1', (64, S1), BF16)
    inp('td0T', (N0, 2 * CD))
    inp('sd0w', (128, S0, CD))
    inp('td1T', (N1, 2 * CD))
    inp('sd1w', (128, 2, CD))
    inp('identb', (128, 128), BF16)
    inp('identf', (128, 128), F32)
    io['out'] = nc.dram_tensor('out', [2], F32, kind="ExternalOutput").ap()

    def scratch(name, shape, dt=BF16):
        io[name] = nc.dram_tensor(name, list(shape), dt).ap()

    scratch('y1t_d', (C, 24, D, D))
    scratch('y1s_d', (C, 12, D, D))
    scratch('y2t0_d', (6, 128, 64))
    scratch('y2t1_d', (C, 64, 64))
    scratch('y2s0_d', (C, 128, 64))
    scratch('y2s1_d', (96, 64))
    scratch('baug_d', (4, N0))
    scratch('aaug_d', (4, N0 // 2))
    scratch('b1aug_d', (4, N1))
    scratch('a1aug_d', (4, N1 // 2))
    scratch('idx0A_d', (N0 // 4,), I16)
    scratch('idx0B_d', (N0 // 4,), I16)
    scratch('idx1_d', (N1 // 2,), I16)

    with tile.TileContext(nc, trace_sim=False) as tc:
        _kern(tc, io)
    nc.compile()
    return nc


_CACHE = {}


def _program():
    if 'nc' not in _CACHE:
        _CACHE['nc'] = _build_program()
    return _CACHE['nc']


def _bf16(x):
    x = np.ascontiguousarray(x, dtype=np.float32)
    u = x.view(np.uint32)
    r = ((u >> 16) & 1) + np.uint32(0x7fff)
    out = ((u + r) & np.uint32(0xffff0000)).view(np.float32)
    import ml_dtypes
    return out.astype(ml_dtypes.bfloat16)


def _normalize_rows(x):
    n = np.maximum(np.linalg.norm(x.astype(np.float32), axis=-1,
                                  keepdims=True).astype(np.float32),
                   np.float32(1e-8))
    return (x / n).astype(np.float32)


def _host_inputs(canonical_source, canonical_target, src_desc0, tgt_desc0,
                 src_desc1, tgt_desc1):
    w0 = _resize_weights(D, S0)   # [64,16]
    w1 = _resize_weights(D, S1)   # [64,8]
    wdt = np.concatenate([w0, w1], axis=1)               # [64,24]
    identb = _bf16(np.eye(128, dtype=np.float32))
    identf = np.eye(128, dtype=np.float32)

    rho0 = np.array([_rho0(i) for i in range(N0 // 2)])
    rho1 = np.array([_rho1(i) for i in range(N1 // 2)])
    # slot i lands at out[p, j] with i = j*128 + p
    perm0 = rho0.reshape(S0, 128).T    # [128 p, 16 j] -> local row
    perm1 = rho1.reshape(2, 128).T     # [128 p, 2 j]

    in_maps = []
    for core in range(NCORES):
        b, h = divmod(core, 2)
        d0 = _SRC_D0[h]
        wds = np.concatenate([w0[d0:d0 + _SRC_DN, 8 * h:8 * h + 8],
                              w1[d0:d0 + _SRC_DN, 4 * h:4 * h + 4]], axis=1)
        ctv = np.ascontiguousarray(
            canonical_target[b].transpose(1, 0, 2, 3).reshape(64, -1))
        csv = np.ascontiguousarray(
            canonical_source[b][:, d0:d0 + _SRC_DN].transpose(1, 0, 2, 3)
            .reshape(_SRC_DN, -1))
        td0n = _normalize_rows(tgt_desc0[b].reshape(CD, N0).T)
        td1n = _normalize_rows(tgt_desc1[b].reshape(CD, N1).T)
        sd0n = _normalize_rows(
            src_desc0[b].reshape(CD, N0).T[h * 2048:(h + 1) * 2048])
        sd1n = _normalize_rows(
            src_desc1[b].reshape(CD, N1).T[h * 256:(h + 1) * 256])
        m = {
            'ct': _bf16(ctv), 'cs': _bf16(csv),
            'wdt': _bf16(wdt), 'wds': _bf16(wds),
            'wh0': _bf16(w0), 'wh1': _bf16(w1),
            'wwb0': _bf16(w0), 'wwa0': _bf16(2.0 * w0),
            'wwb1': _bf16(w1), 'wwa1': _bf16(2.0 * w1),
            'td0T': np.ascontiguousarray(np.pad(td0n, ((0, 0), (0, CD)))),
            'sd0w': np.ascontiguousarray(sd0n[perm0]),
            'td1T': np.ascontiguousarray(np.pad(td1n, ((0, 0), (0, CD)))),
            'sd1w': np.ascontiguousarray(sd1n[perm1]),
            'identb': identb, 'identf': identf,
        }
        in_maps.append(m)
    return in_maps


def kernel(**inputs):
    inputs = {k: np.asarray(v, dtype=np.float32) for k, v in inputs.items()}
    nc = _program()
    in_maps = _host_inputs(**inputs)
    res = run_bass_kernel_spmd(nc, in_maps, list(range(NCORES)))
    _CACHE['last_res'] = res
    parts = np.stack([np.asarray(res.results[c]['out'])
                      for c in range(NCORES)])
    s0 = parts[:, 0].sum(dtype=np.float64)
    s1 = parts[:, 1].sum(dtype=np.float64)
    l0 = np.float32(1.0) - np.float32(s0 / (B * N0))
    l1 = np.float32(1.0) - np.float32(s1 / (B * N1))
    return np.float32((l0 + l1) / 2.0)


# revision 10
# speedup vs baseline: 1.9759x; 1.0168x over previous
"""Trainium2 Bass kernel for nn_DescriptorContrastiveLoss (optimized).

Contract: kernel(**inputs) takes FULL inputs (as produced by
reference.setup_inputs()) and returns the FULL scalar output.

Sharding: data-parallel over (batch, row-half): core c handles batch c//2,
row-half c%2.  Each core:
  - resizes canonical volumes in bf16 (trilinear + antialias, exact jax
    weights quantized to bf16) with three separable contractions (DRAM
    bounces between stages re-partition the data),
  - computes s[n,m] = 2<a_n,b_m> - |b_m|^2 via K=4 bf16 matmuls,
  - argmax per row via DVE running-max scan + is_lt count (exact
    first-occurrence semantics on the bf16 copy of s),
  - builds gather tables via PE transpose of the fp32 indices and gathers
    matched (host-prenormalized) target descriptors with dma_gather,
  - dots against host-prenormalized source descriptors and reduces.
Host combines the 8 partial sums into the final scalar loss.
"""
import sys

sys.path.insert(0, '/opt/trn_rl_repo')

import numpy as np
from contextlib import ExitStack

import concourse.bass as bass
import concourse.tile as tile
import concourse.bacc as bacc
import concourse.mybir as mybir
from concourse._compat import with_exitstack
from concourse.bass_utils import run_bass_kernel_spmd

F32 = mybir.dt.float32
BF16 = mybir.dt.bfloat16
I16 = mybir.dt.int16
ALU = mybir.AluOpType

B = 4
C = 3
D = 64          # input volume side
S0, S1 = 16, 8  # stage output sides
N0, N1 = S0 ** 3, S1 ** 3   # 4096, 512
CD = 32         # descriptor channels
NCORES = 8

# d-slice of the source volume needed per half (with filter support halo)
_SRC_D0 = {0: 0, 1: 28}
_SRC_DN = 36


def _resize_weights(in_size: int, out_size: int) -> np.ndarray:
    """fp32-faithful replica of jax.image resize weights (triangle kernel,
    antialias=True, translation=0).  Returns [in_size, out_size]."""
    scale = out_size / in_size
    inv_scale = np.float32(1.0 / scale)
    kernel_scale = np.float32(max(1.0 / scale, 1.0))
    sample_f = ((np.arange(out_size, dtype=np.float32) + np.float32(0.5))
                * inv_scale - np.float32(0.5))
    x = np.abs(sample_f[None, :]
               - np.arange(in_size, dtype=np.float32)[:, None]) / kernel_scale
    w = np.maximum(np.float32(0), np.float32(1) - x).astype(np.float32)
    tot = w.sum(axis=0, keepdims=True, dtype=np.float32)
    w = np.where(np.abs(tot) > 1000.0 * float(np.finfo(np.float32).eps),
                 w / np.where(tot != 0, tot, 1), 0).astype(np.float32)
    valid = (sample_f >= -0.5) & (sample_f <= in_size - 0.5)
    return np.where(valid[None, :], w, 0).astype(np.float32)


def _rho0(i):
    """gather slot -> local row, stage 0 (matches tableA/B layout)."""
    half = i // 1024
    i = i % 1024
    q, c = i % 16, i // 16
    return 1024 * half + 128 * (q % 8) + 64 * (q // 8) + c


def _rho1(i):
    """gather slot -> local row, stage 1."""
    q, c = i % 16, i // 16
    return 128 * (q % 2) + 16 * (q // 2) + c


@with_exitstack
def _kern(ctx: ExitStack, tc: tile.TileContext, io: dict):
    nc = tc.nc

    consts = ctx.enter_context(tc.tile_pool(name="consts", bufs=1))
    identb = consts.tile([128, 128], BF16)
    nc.sync.dma_start(identb[:], io['identb'])
    identf = consts.tile([128, 128], F32)
    nc.sync.dma_start(identf[:], io['identf'])

    rw = ctx.enter_context(tc.tile_pool(name="rw", bufs=1))
    wdt_sb = rw.tile([64, 24], BF16)
    nc.sync.dma_start(wdt_sb[:], io['wdt'])
    wds_sb = rw.tile([_SRC_DN, 12], BF16)
    nc.sync.dma_start(wds_sb[:], io['wds'])
    wh0_sb = rw.tile([64, S0], BF16)
    nc.sync.dma_start(wh0_sb[:], io['wh0'])
    wh1_sb = rw.tile([64, S1], BF16)
    nc.sync.dma_start(wh1_sb[:], io['wh1'])
    wwb0_sb = rw.tile([64, S0], BF16)
    nc.sync.dma_start(wwb0_sb[:], io['wwb0'])
    wwa0_sb = rw.tile([64, S0], BF16)
    nc.sync.dma_start(wwa0_sb[:], io['wwa0'])
    wwb1_sb = rw.tile([64, S1], BF16)
    nc.sync.dma_start(wwb1_sb[:], io['wwb1'])
    wwa1_sb = rw.tile([64, S1], BF16)
    nc.sync.dma_start(wwa1_sb[:], io['wwa1'])

    # augmented operands (SBUF), loaded from the DRAM staging buffers
    augp = ctx.enter_context(tc.tile_pool(name="aug", bufs=1))
    baug = augp.tile([4, N0], BF16)
    aaug = augp.tile([4, N0 // 2], BF16)
    b1aug = augp.tile([4, N1], BF16)
    a1aug = augp.tile([4, N1 // 2], BF16)
    nc.vector.memset(aaug[0:1, :], 1.0)
    nc.vector.memset(a1aug[0:1, :], 1.0)

    # descriptor tiles (loaded early, consumed by phase G)
    gathp = ctx.enter_context(tc.tile_pool(name="gath", bufs=1))
    sd0w = gathp.tile([128, S0, CD], F32)
    nc.sync.dma_start(sd0w[:], io['sd0w'])
    sd1w = gathp.tile([128, 2, CD], F32)
    nc.sync.dma_start(sd1w[:], io['sd1w'])
    gd0 = gathp.tile([128, S0, 2 * CD], F32)
    gd1 = gathp.tile([128, 2, 2 * CD], F32)
    table0 = gathp.tile([128, 128], I16)
    table1 = gathp.tile([128, 16], I16)
    idx0_f = gathp.tile([128, 16], F32)
    idx1_f = gathp.tile([128, 2], F32)
    cs01 = gathp.tile([128, 2], F32)
    ones_sb = gathp.tile([128, 1], F32)
    nc.vector.memset(ones_sb[:], 1.0)

    baug_d, aaug_d = io['baug_d'], io['aaug_d']
    b1aug_d, a1aug_d = io['b1aug_d'], io['a1aug_d']

    # ---------------- Phase R: resize ----------------
    with tc.tile_pool(name="l1in", bufs=1) as l1p, \
         tc.tile_pool(name="l1out", bufs=2) as l1o, \
         tc.tile_pool(name="y1", bufs=1) as y1p:

        ct_sb = l1p.tile([64, C * D * D], BF16)
        cs_sb = l1p.tile([_SRC_DN, C * D * D], BF16)
        for k in range(4):
            sl = slice(3072 * k, 3072 * (k + 1))
            nc.sync.dma_start(ct_sb[:, sl], io['ct'][:, sl])
            nc.sync.dma_start(cs_sb[:, sl], io['cs'][:, sl])

        # y1[h, c, do-slot, w]: slots 0:24 target (16 st0 + 8 st1),
        # 24:36 source (8 st0 + 4 st1)
        y1 = y1p.tile([64, C, 36, 64], BF16)
        y1t_d = io['y1t_d']   # [C, 24, 64, 64] (c, do, h, w)
        y1s_d = io['y1s_d']   # [C, 12, 64, 64]

        # L1: contract d. 6 chunks of 2048 cols (c, 32 h-rows); t rows 0:24,
        # s rows 32:44 of one PSUM tile.
        with tc.tile_pool(name="psl1", bufs=2, space="PSUM") as psl1:
            for k in range(6):
                p1 = psl1.tile([44, 2048], F32, tag="p1")
                for j in range(4):
                    sl = slice(2048 * k + 512 * j, 2048 * k + 512 * (j + 1))
                    nc.tensor.matmul(p1[0:24, 512 * j:512 * (j + 1)],
                                     wdt_sb[:], ct_sb[:, sl],
                                     start=True, stop=True)
                    nc.tensor.matmul(p1[32:44, 512 * j:512 * (j + 1)],
                                     wds_sb[:], cs_sb[:, sl],
                                     start=True, stop=True)
                s1t = l1o.tile([44, 2048], BF16, tag="s1t")
                nc.scalar.copy(s1t[:], p1[:])
                c, hlo = k // 2, 32 * (k % 2)
                sv = s1t[0:24, :].rearrange("p (h w) -> p h w", h=32)
                nc.sync.dma_start(y1t_d[c, :, hlo:hlo + 32, :], sv)
                sv2 = s1t[32:44, :].rearrange("p (h w) -> p h w", h=32)
                nc.sync.dma_start(y1s_d[c, :, hlo:hlo + 32, :], sv2)

        # reload re-partitioned (h in partitions), per c
        for c in range(C):
            nc.gpsimd.dma_start(
                y1[:, c, 0:24, :],
                y1t_d[c].rearrange("do h w -> h do w"))
            nc.scalar.dma_start(
                y1[:, c, 24:36, :],
                y1s_d[c].rearrange("do h w -> h do w"))

        # L2 (contract h) + L3 (contract w) + stage-1 distance pass S1.
        with tc.tile_pool(name="l2o", bufs=1) as l2o, \
             tc.tile_pool(name="l3in", bufs=2) as l3in, \
             tc.tile_pool(name="l3t", bufs=2) as l3t, \
             tc.tile_pool(name="sq", bufs=1) as sqp, \
             tc.tile_pool(name="s1sb", bufs=2) as s1sbp, \
             tc.tile_pool(name="scn", bufs=2) as scnp, \
             tc.tile_pool(name="psl2", bufs=1, space="PSUM") as psl2, \
             tc.tile_pool(name="pstr", bufs=1, space="PSUM") as pstr, \
             tc.tile_pool(name="psl3", bufs=2, space="PSUM") as psl3, \
             tc.tile_pool(name="pss1", bufs=1, space="PSUM") as pss1:

            y2t0_d = io['y2t0_d']   # [6, 128, 64] block=(c,do-par), row=(do%8,ho)
            y2t1_d = io['y2t1_d']   # [3, 64, 64]  block=c, row=(do,ho)
            y2s0_d = io['y2s0_d']   # [3, 128, 64] block=c, row=(do,ho)
            y2s1_d = io['y2s1_d']   # [96, 64]     row=(c,do,ho)

            for c in range(C):
                p2 = psl2.tile([80, 1024], F32, tag="p2")
                for j in range(2):
                    nc.tensor.matmul(p2[0:16, 512 * j:512 * (j + 1)],
                                     wh0_sb[:],
                                     y1[:, c, 0:16, :].rearrange(
                                         "h do w -> h (do w)")[:,
                                         512 * j:512 * (j + 1)],
                                     start=True, stop=True)
                nc.tensor.matmul(p2[32:40, 0:512], wh1_sb[:],
                                 y1[:, c, 16:24, :].rearrange(
                                     "h do w -> h (do w)"),
                                 start=True, stop=True)
                nc.tensor.matmul(p2[64:80, 0:512], wh0_sb[:],
                                 y1[:, c, 24:32, :].rearrange(
                                     "h do w -> h (do w)"),
                                 start=True, stop=True)
                p2b = psl2.tile([8, 256], F32, tag="p2b")
                nc.tensor.matmul(p2b[:], wh1_sb[:],
                                 y1[:, c, 32:36, :].rearrange(
                                     "h do w -> h (do w)"),
                                 start=True, stop=True)
                s2c = l2o.tile([80, 1024], BF16, tag="s2c")
                nc.scalar.copy(s2c[:], p2[:])
                s2b = l2o.tile([8, 256], BF16, tag="s2b")
                nc.scalar.copy(s2b[:], p2b[:])
                # scatter to y2 DRAM blocks (block row = (do,ho) raster)
                t0v = s2c[0:16, :].rearrange("ho (do w) -> ho do w", do=16)
                nc.sync.dma_start(
                    y2t0_d[2 * c:2 * c + 2].rearrange(
                        "g (do ho) w -> ho (g do) w", do=8),
                    t0v)
                t1v = s2c[32:40, 0:512].rearrange("ho (do w) -> ho do w", do=8)
                nc.sync.dma_start(
                    y2t1_d[c].rearrange("(do ho) w -> ho do w", do=8), t1v)
                s0v = s2c[64:80, 0:512].rearrange("ho (do w) -> ho do w", do=8)
                nc.sync.dma_start(
                    y2s0_d[c].rearrange("(do ho) w -> ho do w", do=8), s0v)
                s1v = s2b[:].rearrange("ho (do w) -> ho do w", do=4)
                nc.sync.dma_start(
                    y2s1_d[32 * c:32 * (c + 1), :].rearrange(
                        "(do ho) w -> ho do w", do=4), s1v)

            # ---- L3 target stage1 (3 chunks of 64 rows, one c each) + b2
            b1sq = sqp.tile([64, S1], F32, tag="b1sq")
            for c in range(C):
                lt = l3in.tile([64, 64], BF16, tag="lt1")
                nc.gpsimd.dma_start(lt[:], y2t1_d[c])
                ptr = pstr.tile([64, 128], BF16, tag="ptr")
                nc.tensor.transpose(ptr[:, 0:64], lt[:], identb[0:64, 0:64])
                tr = l3t.tile([64, 64], BF16, tag="tr1")
                nc.scalar.copy(tr[:], ptr[:, 0:64])
                p3 = psl3.tile([128, S0], F32, tag="p3")
                nc.tensor.matmul(p3[0:64, 0:S1], tr[:], wwb1_sb[:],
                                 start=True, stop=True)
                s3t = l3t.tile([64, S1], BF16, tag="s3t1")
                nc.scalar.copy(s3t[:], p3[0:64, 0:S1])
                nc.sync.dma_start(
                    b1aug_d[1 + c:2 + c, :].rearrange(
                        "one (p w) -> (one p) w", p=64),
                    s3t[:])
                if c == 0:
                    nc.vector.tensor_mul(b1sq[:], s3t[:], s3t[:])
                else:
                    sq2 = sqp.tile([64, S1], F32, tag="b1sq2")
                    nc.vector.tensor_mul(sq2[:], s3t[:], s3t[:])
                    nc.vector.tensor_add(b1sq[:], b1sq[:], sq2[:])
            nc.vector.tensor_scalar_mul(b1sq[:], b1sq[:], -1.0)
            b1sqb = sqp.tile([64, S1], BF16, tag="b1sqb")
            nc.vector.tensor_copy(b1sqb[:], b1sq[:])
            nc.sync.dma_start(
                b1aug_d[0:1, :].rearrange("one (p w) -> (one p) w", p=64),
                b1sqb[:])
            nc.scalar.dma_start(b1aug[:], b1aug_d)

            # ---- L3 source stage1 (1 chunk of 96 rows, 3 c's)
            lt96 = l3in.tile([96, 64], BF16, tag="lt96")
            nc.gpsimd.dma_start(lt96[:], y2s1_d)
            ptr = pstr.tile([64, 128], BF16, tag="ptr")
            nc.tensor.transpose(ptr[:, 0:96], lt96[:], identb[0:96, 0:96])
            tr = l3t.tile([64, 96], BF16, tag="trs1")
            nc.scalar.copy(tr[:], ptr[:, 0:96])
            p3 = psl3.tile([128, S0], F32, tag="p3")
            nc.tensor.matmul(p3[0:96, 0:S1], tr[:], wwa1_sb[:],
                             start=True, stop=True)
            s3a1 = l3t.tile([96, S1], BF16, tag="s3a1")
            nc.scalar.copy(s3a1[:], p3[0:96, 0:S1])
            nc.sync.dma_start(
                a1aug_d[1:4, :].rearrange("c (p w) -> (c p) w", p=32),
                s3a1[:])
            nc.scalar.dma_start(a1aug[1:4, :], a1aug_d[1:4, :])

            # ---- S1: stage-1 distances + scan argmax (overlaps L3 t0/s0)
            for T in range(2):
                ps1 = pss1.tile([128, 512], F32, tag="ps1")
                nc.tensor.matmul(ps1[:], a1aug[:, 128 * T:128 * (T + 1)],
                                 b1aug[:], start=True, stop=True)
                s1sb = s1sbp.tile([128, 512], BF16, tag="s1sb")
                nc.scalar.copy(s1sb[:], ps1[:])
                t81 = scnp.tile([128, 8], BF16, tag="t81")
                nc.vector.max(t81[:], s1sb[:])
                i81 = scnp.tile([128, 8], mybir.dt.uint32, tag="i81")
                nc.vector.max_index(i81[:], t81[:], s1sb[:])
                nc.vector.tensor_copy(idx1_f[:, T:T + 1], i81[:, 0:1])

            # stage-1 gather table: transpose idx -> [2,128] -> i16 -> DRAM
            # DRAM layout = table layout: flat = g8*32 + t*16 + c
            ptr1 = pstr.tile([64, 128], F32, tag="ptr1f")
            nc.tensor.transpose(ptr1[0:2, :], idx1_f[:], identf[:])
            tr1c = l3t.tile([2, 128], I16, tag="tr1c")
            nc.vector.tensor_copy(tr1c[:], ptr1[0:2, :])
            nc.sync.dma_start(
                io['idx1_d'].rearrange("(g8 t c) -> t g8 c", t=2, c=16),
                tr1c[:].rearrange("t (g8 c) -> t g8 c", c=16))
            tb1v = io['idx1_d'].rearrange("(q c) -> q c", c=16)
            for g in range(8):
                nc.gpsimd.dma_start(table1[16 * g:16 * (g + 1), :], tb1v)
            nc.gpsimd.dma_gather(
                out_ap=gd1[:], in_ap=io['td1T'], idxs_ap=table1[:],
                num_idxs=N1 // 2, num_idxs_reg=N1 // 2, elem_size=2 * CD,
                single_packet=False)

            # ---- L3 target stage0 (6 blocks) + b2
            b2sq = sqp.tile([128, S0, 2], F32, tag="b2sq")
            for j in range(6):
                c = j // 2
                par = j % 2
                lt = l3in.tile([128, 64], BF16, tag="lt0")
                nc.gpsimd.dma_start(lt[:], y2t0_d[j])
                ptr = pstr.tile([64, 128], BF16, tag="ptr")
                nc.tensor.transpose(ptr[:], lt[:], identb[:])
                tr = l3t.tile([64, 128], BF16, tag="tr0")
                nc.scalar.copy(tr[:], ptr[:])
                p3 = psl3.tile([128, S0], F32, tag="p3")
                nc.tensor.matmul(p3[:], tr[:], wwb0_sb[:],
                                 start=True, stop=True)
                s3t = l3t.tile([128, S0], BF16, tag="s3t0")
                nc.scalar.copy(s3t[:], p3[:])
                nc.sync.dma_start(
                    baug_d[1 + c:2 + c, 2048 * par:2048 * (par + 1)].rearrange(
                        "one (p w) -> (one p) w", p=128),
                    s3t[:])
                if c == 0:
                    nc.vector.tensor_mul(b2sq[:, :, par], s3t[:], s3t[:])
                else:
                    sq2 = sqp.tile([128, S0], F32, tag="b2sq2")
                    nc.vector.tensor_mul(sq2[:], s3t[:], s3t[:])
                    nc.vector.tensor_add(b2sq[:, :, par], b2sq[:, :, par],
                                         sq2[:])
            for par in range(2):
                nc.vector.tensor_scalar_mul(b2sq[:, :, par], b2sq[:, :, par],
                                            -1.0)
                b2b = sqp.tile([128, S0], BF16, tag="b2b")
                nc.vector.tensor_copy(b2b[:], b2sq[:, :, par])
                nc.sync.dma_start(
                    baug_d[0:1, 2048 * par:2048 * (par + 1)].rearrange(
                        "one (p w) -> (one p) w", p=128),
                    b2b[:])
            nc.scalar.dma_start(baug[:], baug_d)

            # ---- L3 source stage0 (3 blocks)
            for c in range(C):
                lt = l3in.tile([128, 64], BF16, tag="lt0")
                nc.gpsimd.dma_start(lt[:], y2s0_d[c])
                ptr = pstr.tile([64, 128], BF16, tag="ptr")
                nc.tensor.transpose(ptr[:], lt[:], identb[:])
                tr = l3t.tile([64, 128], BF16, tag="trs0")
                nc.scalar.copy(tr[:], ptr[:])
                p3 = psl3.tile([128, S0], F32, tag="p3")
                nc.tensor.matmul(p3[:], tr[:], wwa0_sb[:],
                                 start=True, stop=True)
                s3a = l3t.tile([128, S0], BF16, tag="s3a0")
                nc.scalar.copy(s3a[:], p3[:])
                nc.sync.dma_start(
                    aaug_d[1 + c:2 + c, :].rearrange(
                        "one (p w) -> (one p) w", p=128),
                    s3a[:])
            nc.scalar.dma_start(aaug[1:4, :], aaug_d[1:4, :])

    # ---------------- Phase S0: distances + scan argmax ----------------
    with tc.tile_pool(name="ssb", bufs=2) as ssbp, \
         tc.tile_pool(name="scn0", bufs=2) as scnp, \
         tc.tile_pool(name="psA", bufs=1, space="PSUM") as psa, \
         tc.tile_pool(name="psB", bufs=1, space="PSUM") as psb, \
         tc.tile_pool(name="pstr0", bufs=1, space="PSUM") as pstr0:

        def build_table0(half):
            """transpose idx cols -> DRAM (in table layout) -> table0."""
            ptrh = pstr0.tile([8, 128], F32, tag="ptrh")
            nc.tensor.transpose(ptrh[:], idx0_f[:, 8 * half:8 * (half + 1)],
                                identf[:])
            trc = scnp.tile([8, 128], I16, tag="trc" + str(half))
            nc.vector.tensor_copy(trc[:], ptrh[:])
            dv = io['idx0A_d'] if half == 0 else io['idx0B_d']
            # table layout: flat = g2*512 + t*64 + c  (q = g2*8 + t)
            nc.sync.dma_start(
                dv.rearrange("(g2 t c) -> t g2 c", t=8, c=64),
                trc[:].rearrange("t (g2 c) -> t g2 c", c=64))
            tbv = dv.rearrange("(q c) -> q c", c=64)
            for g in range(8):
                nc.gpsimd.dma_start(
                    table0[16 * g:16 * (g + 1), 64 * half:64 * (half + 1)],
                    tbv)
            nc.gpsimd.dma_gather(
                out_ap=gd0[:, 8 * half:8 * (half + 1), :], in_ap=io['td0T'],
                idxs_ap=table0[:, 64 * half:64 * (half + 1)],
                num_idxs=N0 // 4, num_idxs_reg=N0 // 4, elem_size=2 * CD,
                single_packet=False)

        for T in range(16):
            lhs = aaug[:, 128 * T:128 * (T + 1)]
            s_sb = ssbp.tile([128, 4096], BF16, tag="ssb")
            for q in range(4):
                pool = psa if q % 2 == 0 else psb
                ph = pool.tile([128, 1024], F32, tag="p" + str(q % 2))
                for j in range(2):
                    nc.tensor.matmul(ph[:, 512 * j:512 * (j + 1)], lhs,
                                     baug[:, 1024 * q + 512 * j:
                                          1024 * q + 512 * (j + 1)],
                                     start=True, stop=True)
                nc.scalar.copy(s_sb[:, 1024 * q:1024 * (q + 1)], ph[:])
            sv = s_sb[:].rearrange("p (blk r) -> p blk r", blk=32)
            f = scnp.tile([128, 32, 64], BF16, tag="f0")
            nc.vector.tensor_tensor(f[:], sv[:, :, 0:64], sv[:, :, 64:128],
                                    ALU.max)
            nc.vector.tensor_tensor(f[:, :, 0:32], f[:, :, 0:32],
                                    f[:, :, 32:64], ALU.max)
            nc.vector.tensor_tensor(f[:, :, 0:16], f[:, :, 0:16],
                                    f[:, :, 16:32], ALU.max)
            nc.vector.tensor_tensor(f[:, :, 0:8], f[:, :, 0:8],
                                    f[:, :, 8:16], ALU.max)
            t8 = scnp.tile([128, 8], BF16, tag="t8")
            nc.vector.max(t8[:], f[:, :, 0:8])
            i8 = scnp.tile([128, 8], mybir.dt.uint32, tag="i8")
            nc.vector.max_index(i8[:], t8[:], s_sb[:])
            nc.vector.tensor_copy(idx0_f[:, T:T + 1], i8[:, 0:1])
            if T == 7:
                build_table0(0)
        build_table0(1)

        # ---------------- Phase G: cosine + reduce ----------------
        cw = scnp  # reuse pool for small cosine tiles
        prod = cw.tile([128, S0, CD], F32, tag="prod")
        nc.vector.tensor_mul(prod[:, 0:8, :], sd0w[:, 0:8, :],
                             gd0[:, 0:8, 0:CD])
        nc.vector.tensor_mul(prod[:, 8:16, :], sd0w[:, 8:16, :],
                             gd0[:, 8:16, 0:CD])
        cos0 = cw.tile([128, S0], F32, tag="cos0")
        nc.vector.reduce_sum(cos0[:], prod[:], axis=mybir.AxisListType.X)
        nc.vector.reduce_sum(cs01[:, 0:1], cos0[:], axis=mybir.AxisListType.X)
        prod1 = cw.tile([128, 2, CD], F32, tag="prod1")
        nc.vector.tensor_mul(prod1[:], sd1w[:], gd1[:, :, 0:CD])
        cos1 = cw.tile([128, 2], F32, tag="cos1")
        nc.vector.reduce_sum(cos1[:], prod1[:], axis=mybir.AxisListType.X)
        nc.vector.reduce_sum(cs01[:, 1:2], cos1[:], axis=mybir.AxisListType.X)

        pf = pstr0.tile([2, 1], F32, tag="pf")
        nc.tensor.matmul(pf[:], cs01[:], ones_sb[:], start=True, stop=True)
        of = cw.tile([2, 1], F32, tag="of")
        nc.scalar.copy(of[:], pf[:])
        nc.sync.dma_start(io['out'].rearrange("(a one) -> a one", one=1),
                          of[:])


def _build_program():
    nc = bacc.Bacc("TRN2", target_bir_lowering=False, debug=False,
                   enable_asserts=True, num_devices=NCORES)
    io = {}

    def inp(name, shape, dt=F32):
        io[name] = nc.dram_tensor(name, list(shape), dt,
                                  kind="ExternalInput").ap()

    inp('ct', (64, C * D * D), BF16)
    inp('cs', (_SRC_DN, C * D * D), BF16)
    inp('wdt', (64, 24), BF16)
    inp('wds', (_SRC_DN, 12), BF16)
    inp('wh0', (64, S0), BF16)
    inp('wh1', (64, S1), BF16)
    inp('wwb0', (64, S0), BF16)
    inp('wwa0', (64, S0), BF16)
    inp('wwb1', (64, S1), BF16)
    inp('wwa1', (64, S1), BF16)
    inp('td0T', (N0, 2 * CD))
    inp('sd0w', (128, S0, CD))
    inp('td1T', (N1, 2 * CD))
    inp('sd1w', (128, 2, CD))
    inp('identb', (128, 128), BF16)
    inp('identf', (128, 128), F32)
    io['out'] = nc.dram_tensor('out', [2], F32, kind="ExternalOutput").ap()

    def scratch(name, shape, dt=BF16):
        io[name] = nc.dram_tensor(name, list(shape), dt).ap()

    scratch('y1t_d', (C, 24, D, D))
    scratch('y1s_d', (C, 12, D, D))
    scratch('y2t0_d', (6, 128, 64))
    scratch('y2t1_d', (C, 64, 64))
    scratch('y2s0_d', (C, 128, 64))
    scratch('y2s1_d', (96, 64))
    scratch('baug_d', (4, N0))
    scratch('aaug_d', (4, N0 // 2))
    scratch('b1aug_d', (4, N1))
    scratch('a1aug_d', (4, N1 // 2))
    scratch('idx0A_d', (N0 // 4,), I16)
    scratch('idx0B_d', (N0 // 4,), I16)
    scratch('idx1_d', (N1 // 2,), I16)

    with tile.TileContext(nc, trace_sim=False) as tc:
        _kern(tc, io)
    nc.compile()
    return nc


_CACHE = {}


def _program():
    if 'nc' not in _CACHE:
        _CACHE['nc'] = _build_program()
    return _CACHE['nc']


def _bf16(x):
    x = np.ascontiguousarray(x, dtype=np.float32)
    u = x.view(np.uint32)
    r = ((u >> 16) & 1) + np.uint32(0x7fff)
    out = ((u + r) & np.uint32(0xffff0000)).view(np.float32)
    import ml_dtypes
    return out.astype(ml_dtypes.bfloat16)


def _normalize_rows(x):
    n = np.maximum(np.linalg.norm(x.astype(np.float32), axis=-1,
                                  keepdims=True).astype(np.float32),
                   np.float32(1e-8))
    return (x / n).astype(np.float32)


def _host_inputs(canonical_source, canonical_target, src_desc0, tgt_desc0,
                 src_desc1, tgt_desc1):
    w0 = _resize_weights(D, S0)   # [64,16]
    w1 = _resize_weights(D, S1)   # [64,8]
    wdt = np.concatenate([w0, w1], axis=1)               # [64,24]
    identb = _bf16(np.eye(128, dtype=np.float32))
    identf = np.eye(128, dtype=np.float32)

    rho0 = np.array([_rho0(i) for i in range(N0 // 2)])
    rho1 = np.array([_rho1(i) for i in range(N1 // 2)])
    # slot i lands at out[p, j] with i = j*128 + p
    perm0 = rho0.reshape(S0, 128).T    # [128 p, 16 j] -> local row
    perm1 = rho1.reshape(2, 128).T     # [128 p, 2 j]

    in_maps = []
    for core in range(NCORES):
        b, h = divmod(core, 2)
        d0 = _SRC_D0[h]
        wds = np.concatenate([w0[d0:d0 + _SRC_DN, 8 * h:8 * h + 8],
                              w1[d0:d0 + _SRC_DN, 4 * h:4 * h + 4]], axis=1)
        ctv = np.ascontiguousarray(
            canonical_target[b].transpose(1, 0, 2, 3).reshape(64, -1))
        csv = np.ascontiguousarray(
            canonical_source[b][:, d0:d0 + _SRC_DN].transpose(1, 0, 2, 3)
            .reshape(_SRC_DN, -1))
        td0n = _normalize_rows(tgt_desc0[b].reshape(CD, N0).T)
        td1n = _normalize_rows(tgt_desc1[b].reshape(CD, N1).T)
        sd0n = _normalize_rows(
            src_desc0[b].reshape(CD, N0).T[h * 2048:(h + 1) * 2048])
        sd1n = _normalize_rows(
            src_desc1[b].reshape(CD, N1).T[h * 256:(h + 1) * 256])
        m = {
            'ct': _bf16(ctv), 'cs': _bf16(csv),
            'wdt': _bf16(wdt), 'wds': _bf16(wds),
            'wh0': _bf16(w0), 'wh1': _bf16(w1),
            'wwb0': _bf16(w0), 'wwa0': _bf16(2.0 * w0),
            'wwb1': _bf16(w1), 'wwa1': _bf16(2.0 * w1),
            'td0T': np.ascontiguousarray(np.pad(td0n, ((0, 0), (0, CD)))),
            'sd0w': np.ascontiguousarray(sd0n[perm0]),
            'td1T': np.ascontiguousarray(np.pad(td1n, ((0, 0), (0, CD)))),
            'sd1w': np.ascontiguousarray(sd1n[perm1]),
            'identb': identb, 'identf': identf,
        }
        in_maps.append(m)
    return in_maps


def kernel(**inputs):
    inputs = {k: np.asarray(v, dtype=np.float32) for k, v in inputs.items()}
    nc = _program()
    in_maps = _host_inputs(**inputs)
    res = run_bass_kernel_spmd(nc, in_maps, list(range(NCORES)))
    _CACHE['last_res'] = res
    parts = np.stack([np.asarray(res.results[c]['out'])
                      for c in range(NCORES)])
    s0 = parts[:, 0].sum(dtype=np.float64)
    s1 = parts[:, 1].sum(dtype=np.float64)
    l0 = np.float32(1.0) - np.float32(s0 / (B * N0))
    l1 = np.float32(1.0) - np.float32(s1 / (B * N1))
    return np.float32((l0 + l1) / 2.0)
